# revision 10
# baseline (speedup 1.0000x reference)
"""Two-layer GAT on 8 Trainium2 NeuronCores.

Strategy (row-sharded dense attention):
  - Nodes (= rows of the dense NxN attention matrix) are sharded across the 8
    cores; core c owns rows [c*S, (c+1)*S), S = N/8.
  - The adjacency structure is fixed by edge_list, so the 0/1 mask is built on
    the host (fp8, transposed [t, s] layout) and streamed from HBM; it is
    reused by all 4 heads of layer 1 and by layer 2.
  - exp(leaky_relu(as+at)) = max(exp(as+at), exp(0.2*(as+at))): two ACT Exp
    passes with per-partition bias (at columns are produced by the h = x@W.T
    matmul itself, via extra host-precomputed weight columns W.T@a).
  - Attention output is accumulated transposed: outT[f, s] = sum_j
    (h|1)_j.T @ PT_j, with a ones column giving the softmax denominator Z
    for free (softmax without max subtraction - the logit range is small).
  - Duplicate edges (where the reference scatter-add sums e values) are
    excluded from the mask and corrected exactly with a small gather
    (indirect DMA) + rank-limited correction matmul.
  - One small AllGather ([S, 20] per core) carries layer-2 h2 / alpha columns
    between the layers; everything else is local.
"""

import math
from dataclasses import dataclass

import ml_dtypes
import numpy as np

import concourse.bass as bass
import concourse.mybir as mybir
import concourse.tile as tile
from concourse import bacc
from concourse.bass_utils import run_bass_kernel_spmd
from concourse.masks import make_identity

F32 = mybir.dt.float32
F32R = mybir.dt.float32r
FP8 = mybir.dt.float8e4
I32 = mybir.dt.int32
AF = mybir.ActivationFunctionType
OP = mybir.AluOpType
AX = mybir.AxisListType

P = 128


@dataclass(frozen=True)
class GATConfig:
    n: int = 8192          # nodes
    f_in: int = 512        # input features
    nhid: int = 64         # per-head hidden
    heads: int = 4
    nclass: int = 16
    ncores: int = 8
    dup_rounds: int = 1    # ceil(max dup edges per core / 128)

    @property
    def s(self):           # rows per core
        return self.n // self.ncores

    @property
    def jt(self):          # 128-row t tiles
        return self.n // P

    @property
    def sc(self):          # 128-row chunks of own block
        return self.s // P

    @property
    def kt1(self):         # k tiles of f_in
        return self.f_in // P

    @property
    def fcat(self):        # concat feature dim after layer 1
        return self.nhid * self.heads

    @property
    def kt2(self):
        return self.fcat // P

    @property
    def hstride(self):     # per-head column stride in h1buf: h|1|as|at|0.2at
        return self.nhid + 4

    @property
    def w1cols(self):
        return self.hstride * self.heads

    # layer-2 psum/payload columns: h2(nclass) | ones | at | 0.2at | as
    @property
    def w2cols(self):
        return self.nclass + 4


def _ts(i, sz):
    return slice(i * sz, (i + 1) * sz)


def _halves(s):
    return [slice(h0, min(h0 + 512, s)) for h0 in range(0, s, 512)]


def build_gat_kernel(tc, cfg: GATConfig, io):
    """Emit the GAT program. io: dict of DRAM APs (inputs+outputs)."""
    nc = tc.nc
    n, s, jt, sc = cfg.n, cfg.s, cfg.jt, cfg.sc
    hs, nh, nheads = cfg.hstride, cfg.nhid, cfg.heads
    ncl = cfg.nclass
    MCH = 2  # mask j-tiles per DMA chunk
    njc = jt // MCH

    xT, xTown = io["xT"], io["xTown"]
    w1t, w2t, maskT = io["w1t"], io["w2t"], io["maskT"]
    tdup, sdup, dmult, sonehot = io["tdup"], io["sdup"], io["dmult"], io["sonehot"]
    out_dram = io["out"]

    with (
        tc.tile_pool(name="persist", bufs=1) as sb1,
        tc.tile_pool(name="stream", bufs=2) as sb_stream,
        tc.tile_pool(name="etile", bufs=2) as sb_e,
        tc.tile_pool(name="pttile", bufs=2) as sb_pt,
        tc.tile_pool(name="misc", bufs=2) as sb_misc,
        tc.tile_pool(name="ps_small", bufs=2, space="PSUM") as ps_small,
        tc.tile_pool(name="ps_big", bufs=2, space="PSUM") as ps_big,
        tc.tile_pool(name="dram", bufs=1, space="DRAM") as dram,
    ):
        # ---------------- persistent tiles ----------------
        w1t_sb = sb1.tile([P, cfg.kt1, cfg.w1cols], F32R, tag="w1t_sb")
        nc.sync.dma_start(
            w1t_sb[:], w1t[:].rearrange("(k p) c -> p k c", p=P))
        w2t_sb = sb1.tile([P, cfg.kt2, cfg.w2cols], F32R, tag="w2t_sb")
        nc.sync.dma_start(
            w2t_sb[:], w2t[:].rearrange("(k p) c -> p k c", p=P))

        h1buf = sb1.tile([P, jt, cfg.w1cols], F32R, tag="h1buf")
        hl2T = sb1.tile([P, cfg.kt2, s], F32R, tag="hl2T")
        g_all = sb1.tile([P, jt, cfg.w2cols], F32R, tag="g_all")

        ones1 = sb1.tile([1, P], F32R, tag="ones1")
        nc.vector.memset(ones1[:].bitcast(F32), 1.0)
        ident = sb1.tile([P, P], F32, tag="ident")
        make_identity(nc, ident[:])

        # dup inputs resident
        tdup_sb = sb1.tile([P, cfg.dup_rounds, 1], I32, tag="tdup_sb")
        sdup_sb = sb1.tile([P, cfg.dup_rounds, 1], I32, tag="sdup_sb")
        dmult_sb = sb1.tile([P, cfg.dup_rounds, 1], F32, tag="dmult_sb")
        nc.sync.dma_start(tdup_sb[:], tdup[:].rearrange("(r p) c -> p r c", p=P))
        nc.sync.dma_start(sdup_sb[:], sdup[:].rearrange("(r p) c -> p r c", p=P))
        nc.sync.dma_start(dmult_sb[:], dmult[:].rearrange("(r p) c -> p r c", p=P))
        soh_sb = sb1.tile([P, cfg.dup_rounds, s], F32R, tag="soh_sb")
        nc.sync.dma_start(soh_sb[:], sonehot[:].rearrange("(r p) c -> p r c", p=P))

        h1dram = dram.tile([n, cfg.w1cols], F32R)
        payload_dram = dram.tile([s, cfg.w2cols], F32R)
        g_dram = dram.tile([n, cfg.w2cols], F32R)

        # ---------------- phase A: h1 = x @ W1ext (all rows, all heads) ----
        for j in range(jt):
            xtc = sb_stream.tile([P, cfg.kt1, P], F32R, tag="xtc")
            nc.sync.dma_start(
                xtc[:], xT[:, _ts(j, P)].rearrange("(k p) m -> p k m", p=P))
            psA = ps_small.tile([P, cfg.w1cols], F32, tag="small")
            for k in range(cfg.kt1):
                nc.tensor.matmul(
                    psA[:], xtc[:, k, :],
                    w1t_sb[:, k, :],
                    start=(k == 0), stop=(k == cfg.kt1 - 1))
            nc.scalar.copy(h1buf[:, j, :], psA[:])
        # ones columns
        for h in range(nheads):
            nc.vector.memset(h1buf[:, :, h * hs + nh].bitcast(F32), 1.0)
        # h1 -> DRAM for the dup gathers
        nc.sync.dma_start(
            h1dram[:].rearrange("(j p) c -> p j c", p=P), h1buf[:])

        # dup gathers for layer 1 (rows of h1ext at t and s indices)
        hdup1 = []
        sdup1 = []
        for r in range(cfg.dup_rounds):
            ht = sb1.tile([P, cfg.w1cols], F32R, tag=f"hdup1_{r}")
            nc.gpsimd.indirect_dma_start(
                out=ht[:], out_offset=None, in_=h1dram[:],
                in_offset=bass.IndirectOffsetOnAxis(ap=tdup_sb[:, r, :], axis=0))
            hdup1.append(ht)
            hs_ = sb1.tile([P, cfg.w1cols], F32R, tag=f"sdup1_{r}")
            nc.gpsimd.indirect_dma_start(
                out=hs_[:], out_offset=None, in_=h1dram[:],
                in_offset=bass.IndirectOffsetOnAxis(ap=sdup_sb[:, r, :], axis=0))
            sdup1.append(hs_)

        # alpha_src row [1, s] per head
        asrow = []
        for h in range(nheads):
            psr = ps_small.tile([1, s], F32, tag="small")
            for k in range(cfg.kt1):
                xo = sb_stream.tile([P, s], F32R, tag="xo")
                nc.sync.dma_start(xo[:], xTown[_ts(k, P), :])
                for hsl in _halves(s):
                    nc.tensor.matmul(
                        psr[:, hsl],
                        w1t_sb[:, k, h * hs + nh + 1: h * hs + nh + 2],
                        xo[:, hsl],
                        start=(k == 0), stop=(k == cfg.kt1 - 1))
            ar = sb1.tile([1, s], F32R, tag=f"asrow_{h}", name=f"asrow_{h}")
            nc.scalar.copy(ar[:], psr[:])
            asrow.append(ar)

        # ---------------- layer 1 attention, 2 heads per pass -------------
        def attn_pass(heads_list, lhs_getter, at_getter, at2_getter, asb_list,
                      hdup, sdupt, dup_alpha_s_col, dup_alpha_t_col,
                      dup_lhs_cols, psum_tiles, mask_dtype):
            """One full j sweep accumulating outT for the given heads."""
            npass = len(heads_list)
            for jc in range(njc):
                mtile = sb_stream.tile([P, MCH, s], mask_dtype, tag="mtile")
                nc.sync.dma_start(
                    mtile[:],
                    maskT[_ts(jc, MCH * P), :].rearrange(
                        "(c p) ss -> p c ss", p=P))
                for jj in range(MCH):
                    j = jc * MCH + jj
                    for hi, h in enumerate(heads_list):
                        e1 = sb_e.tile([P, s], F32, tag="e1")
                        nc.scalar.activation(
                            e1[:], asb_list[hi][:], AF.Exp,
                            bias=at_getter(j, h), scale=1.0)
                        e2 = sb_e.tile([P, s], F32, tag="e2")
                        nc.scalar.activation(
                            e2[:], asb_list[hi][:], AF.Exp,
                            bias=at2_getter(j, h), scale=0.2)
                        nc.vector.tensor_max(e1[:], e1[:], e2[:])
                        pt = sb_pt.tile([P, s], F32R, tag="pt")
                        nc.vector.tensor_mul(pt[:], e1[:], mtile[:, jj, :])
                        for hsl in _halves(s):
                            nc.tensor.matmul(
                                psum_tiles[hi][:, hsl],
                                lhs_getter(j, h),
                                pt[:, hsl],
                                start=(j == 0), stop=False)
            # dup corrections close the accumulation group
            for hi, h in enumerate(heads_list):
                for r in range(cfg.dup_rounds):
                    at_d = hdup[r][:, dup_alpha_t_col(h):dup_alpha_t_col(h) + 1]
                    as_d = sdupt[r][:, dup_alpha_s_col(h):dup_alpha_s_col(h) + 1]
                    u = sb_misc.tile([P, 1], F32, tag="u_d")
                    nc.vector.tensor_add(u[:], at_d, as_d)
                    # w8 = 0.8*relu(u)
                    w8 = sb_misc.tile([P, 1], F32, tag="w8_d")
                    nc.vector.tensor_scalar(
                        out=w8[:], in0=u[:], scalar1=0.0, scalar2=0.8,
                        op0=OP.max, op1=OP.mult)
                    # w = (0.2*u + w8) * mult
                    w_ = sb_misc.tile([P, 1], F32, tag="w_d")
                    nc.vector.scalar_tensor_tensor(
                        out=w_[:], in0=u[:], scalar=0.2, in1=w8[:],
                        op0=OP.mult, op1=OP.add)
                    nc.vector.tensor_mul(w_[:], w_[:], dmult_sb[:, r, :])
                    dl = sb_misc.tile([P, 1], F32, tag="dl_d")
                    nc.scalar.activation(dl[:], w_[:], AF.Exp)
                    c0, c1 = dup_lhs_cols(h)
                    hsc = sb_misc.tile([P, c1 - c0], F32R, tag="hsc_d")
                    nc.vector.tensor_scalar_mul(hsc[:], hdup[r][:, c0:c1], dl[:])
                    for hsl in _halves(s):
                        nc.tensor.matmul(
                            psum_tiles[hi][:, hsl],
                            hsc[:],
                            soh_sb[:, r, hsl],
                            start=False, stop=(r == cfg.dup_rounds - 1))

        def build_asb(ar):
            """[1,s] row AP -> [128,s] broadcast tile (PE outer product)."""
            psb = ps_big.tile([P, s], F32, tag="big")
            for hsl in _halves(s):
                nc.tensor.matmul(psb[:, hsl], ones1[:], ar[:, hsl],
                                 start=True, stop=True)
            asb = sb_misc.tile([P, s], F32, tag="asb")
            nc.scalar.copy(asb[:], psb[:])
            return asb

        for pp in range(nheads // 2):
            hpair = [2 * pp, 2 * pp + 1]
            asb_list = [build_asb(asrow[h][0:1, :]) for h in hpair]
            psum_tiles = [
                ps_big.tile([nh + 1, s], F32, tag="big", name=f"attnps_{pp}_{i}")
                for i in range(2)]
            attn_pass(
                hpair,
                lhs_getter=lambda j, h: h1buf[:, j, h * hs: h * hs + nh + 1],
                at_getter=lambda j, h: h1buf[:, j, h * hs + nh + 2: h * hs + nh + 3],
                at2_getter=lambda j, h: h1buf[:, j, h * hs + nh + 3: h * hs + nh + 4],
                asb_list=asb_list,
                hdup=hdup1, sdupt=sdup1,
                dup_alpha_s_col=lambda h: h * hs + nh + 1,
                dup_alpha_t_col=lambda h: h * hs + nh + 2,
                dup_lhs_cols=lambda h: (h * hs, h * hs + nh + 1),
                psum_tiles=psum_tiles, mask_dtype=FP8)

            # normalize + ELU into hl2T (transposed feature-major layout)
            for hi, h in enumerate(hpair):
                otT = sb_misc.tile([nh + 1, s], F32, tag="otT")
                nc.scalar.copy(otT[:], psum_tiles[hi][:])
                rz = sb_misc.tile([1, s], F32R, tag="rz", bufs=1)
                with nc.allow_low_precision(reason="fp22 recip row for PE bcast"):
                    nc.vector.reciprocal(rz[:], otT[nh:nh + 1, :])
                rzb = ps_big.tile([nh, s], F32, tag="big")
                for hsl in _halves(s):
                    nc.tensor.matmul(rzb[:, hsl], ones1[:, :nh],
                                     rz[0:1, hsl], start=True, stop=True)
                xn = sb_misc.tile([nh, s], F32, tag="xn", bufs=1)
                nc.vector.tensor_mul(xn[:], otT[:nh, :], rzb[:])
                t1 = sb_misc.tile([nh, s], F32, tag="t1", bufs=1)
                nc.vector.tensor_single_scalar(t1[:], xn[:], 0.0, OP.min)
                t2 = sb_misc.tile([nh, s], F32, tag="t2", bufs=1)
                nc.scalar.activation(t2[:], t1[:], AF.Exp)
                t4 = sb_misc.tile([nh, s], F32, tag="t4", bufs=1)
                nc.vector.tensor_scalar(
                    out=t4[:], in0=xn[:], scalar1=0.0, scalar2=-1.0,
                    op0=OP.max, op1=OP.add)
                kt = h // 2
                ro = nh * (h % 2)
                nc.vector.tensor_add(
                    hl2T[ro:ro + nh, kt, :], t4[:], t2[:])

        # ---------------- layer 2 build + AllGather ------------------------
        a2srow = sb1.tile([1, s], F32R, tag="a2srow")
        for c in range(sc):
            ps2 = ps_small.tile([P, cfg.w2cols], F32, tag="small")
            for k in range(cfg.kt2):
                nc.tensor.matmul(
                    ps2[:], hl2T[:, k, _ts(c, P)],
                    w2t_sb[:, k, :],
                    start=(k == 0), stop=(k == cfg.kt2 - 1))
            pay = sb_misc.tile([P, cfg.w2cols], F32R, tag="pay")
            nc.scalar.copy(pay[:], ps2[:])
            nc.vector.memset(pay[:, ncl:ncl + 1].bitcast(F32), 1.0)
            nc.sync.dma_start(payload_dram[_ts(c, P), :], pay[:])
            # alpha2_src row segment (col ncl+3)
            nc.sync.dma_start(
                a2srow[0:1, _ts(c, P)], pay[:, ncl + 3:ncl + 4])

        nc.gpsimd.collective_compute(
            "AllGather", OP.bypass,
            replica_groups=[list(range(cfg.ncores))],
            ins=[payload_dram.opt()], outs=[g_dram.opt()])

        nc.sync.dma_start(
            g_all[:], g_dram[:].rearrange("(j p) c -> p j c", p=P))

        hdup2 = []
        sdup2 = []
        for r in range(cfg.dup_rounds):
            ht = sb1.tile([P, cfg.w2cols], F32R, tag=f"hdup2_{r}")
            nc.gpsimd.indirect_dma_start(
                out=ht[:], out_offset=None, in_=g_dram[:],
                in_offset=bass.IndirectOffsetOnAxis(ap=tdup_sb[:, r, :], axis=0))
            hdup2.append(ht)
            hs_ = sb1.tile([P, cfg.w2cols], F32R, tag=f"sdup2_{r}")
            nc.gpsimd.indirect_dma_start(
                out=hs_[:], out_offset=None, in_=g_dram[:],
                in_offset=bass.IndirectOffsetOnAxis(ap=sdup_sb[:, r, :], axis=0))
            sdup2.append(hs_)

        # ---------------- layer 2 attention --------------------------------
        asb2 = [build_asb(a2srow[0:1, :])]
        psum2 = [ps_big.tile([ncl + 1, s], F32, tag="big", name="attnps2")]
        attn_pass(
            [0],
            lhs_getter=lambda j, h: g_all[:, j, 0:ncl + 1],
            at_getter=lambda j, h: g_all[:, j, ncl + 1:ncl + 2],
            at2_getter=lambda j, h: g_all[:, j, ncl + 2:ncl + 3],
            asb_list=asb2,
            hdup=hdup2, sdupt=sdup2,
            dup_alpha_s_col=lambda h: ncl + 3,
            dup_alpha_t_col=lambda h: ncl + 1,
            dup_lhs_cols=lambda h: (0, ncl + 1),
            psum_tiles=psum2, mask_dtype=FP8)

        # ---------------- epilogue: transpose, normalize, log_softmax ------
        otT2 = sb1.tile([ncl + 1, s], F32, tag="otT2")
        nc.scalar.copy(otT2[:], psum2[0][:])
        for c in range(sc):
            pst = ps_small.tile([P, ncl + 1], F32, tag="small")
            nc.tensor.transpose(
                pst[:], otT2[:, _ts(c, P)], ident[0:ncl + 1, 0:ncl + 1])
            rz = sb_misc.tile([P, 1], F32, tag="rz2")
            nc.vector.reciprocal(rz[:], pst[:, ncl:ncl + 1])
            lg = sb_misc.tile([P, ncl], F32, tag="lg")
            nc.vector.tensor_scalar_mul(lg[:], pst[:, 0:ncl], rz[:])
            m = sb_misc.tile([P, 1], F32, tag="m2")
            nc.vector.tensor_reduce(m[:], lg[:], axis=AX.X, op=OP.max)
            negm = sb_misc.tile([P, 1], F32, tag="negm")
            nc.vector.tensor_single_scalar(negm[:], m[:], -1.0, OP.mult)
            exd = sb_misc.tile([P, ncl], F32, tag="exd")
            zs = sb_misc.tile([P, 1], F32, tag="zs")
            nc.scalar.activation(exd[:], lg[:], AF.Exp, bias=negm[:, 0:1],
                                 accum_out=zs[:, 0:1])
            lnz = sb_misc.tile([P, 1], F32, tag="lnz")
            nc.scalar.activation(lnz[:], zs[:], AF.Ln)
            fin = sb_misc.tile([P, ncl], F32, tag="fin")
            nc.vector.tensor_scalar(
                out=fin[:], in0=lg[:], scalar1=m[:, 0:1], scalar2=lnz[:, 0:1],
                op0=OP.subtract, op1=OP.subtract)
            nc.sync.dma_start(out_dram[_ts(c, P), :], fin[:])


# ======================= host side =======================================


def _leaky(x, alpha=0.2):
    return np.where(x > 0, x, alpha * x)


def preprocess(cfg: GATConfig, x, edge_list, W1, a1, W2, a2):
    """Build per-core input maps (numpy only)."""
    n, s = cfg.n, cfg.s
    src = np.asarray(edge_list[0]).astype(np.int64)
    tgt = np.asarray(edge_list[1]).astype(np.int64)
    key = src * n + tgt
    uniq, counts = np.unique(key, return_counts=True)
    us = (uniq // n).astype(np.int32)   # row (softmax) index
    ut = (uniq % n).astype(np.int32)    # col index
    singles = counts == 1
    dups = ~singles

    # transposed mask [t, s]; dup cells excluded (corrected exactly later)
    maskT = np.zeros((n, n), dtype=ml_dtypes.float8_e4m3)
    maskT[ut[singles], us[singles]] = 1.0

    # row coverage check: every row must have at least one edge
    row_deg = np.bincount(us, minlength=n)
    assert row_deg.min() > 0, "empty adjacency row: kernel assumes none"

    d_s, d_t, d_m = us[dups], ut[dups], counts[dups].astype(np.float32)

    x = np.asarray(x, dtype=np.float32)
    xT = np.ascontiguousarray(x.T)

    W1 = np.asarray(W1, dtype=np.float32)
    a1 = np.asarray(a1, dtype=np.float32)
    W2 = np.asarray(W2, dtype=np.float32)
    a2 = np.asarray(a2, dtype=np.float32)

    hs, nh = cfg.hstride, cfg.nhid
    w1t = np.zeros((cfg.f_in, cfg.w1cols), np.float32)
    for h in range(cfg.heads):
        w1t[:, h * hs: h * hs + nh] = W1[h].T
        va_s = W1[h].T @ a1[h, :nh]
        va_t = W1[h].T @ a1[h, nh:]
        w1t[:, h * hs + nh + 1] = va_s
        w1t[:, h * hs + nh + 2] = va_t
        w1t[:, h * hs + nh + 3] = 0.2 * va_t

    ncl = cfg.nclass
    w2t = np.zeros((cfg.fcat, cfg.w2cols), np.float32)
    w2t[:, 0:ncl] = W2.T
    va2_s = W2.T @ a2[:ncl]
    va2_t = W2.T @ a2[ncl:]
    w2t[:, ncl + 1] = va2_t
    w2t[:, ncl + 2] = 0.2 * va2_t
    w2t[:, ncl + 3] = va2_s

    in_maps = []
    max_dups = 0
    for c in range(cfg.ncores):
        blk = slice(c * s, (c + 1) * s)
        mine = (d_s >= c * s) & (d_s < (c + 1) * s)
        max_dups = max(max_dups, int(mine.sum()))
    dup_cap = cfg.dup_rounds * P
    assert max_dups <= dup_cap, f"{max_dups} dups > cap {dup_cap}"

    for c in range(cfg.ncores):
        blk = slice(c * s, (c + 1) * s)
        mine = np.nonzero((d_s >= c * s) & (d_s < (c + 1) * s))[0]
        k = len(mine)
        td = np.zeros((dup_cap, 1), np.int32)
        sd = np.zeros((dup_cap, 1), np.int32)
        dm = np.zeros((dup_cap, 1), np.float32)
        soh = np.zeros((dup_cap, s), np.float32)
        td[:k, 0] = d_t[mine]
        sd[:k, 0] = d_s[mine]
        dm[:k, 0] = d_m[mine]
        soh[np.arange(k), d_s[mine] - c * s] = 1.0
        in_maps.append({
            "xT": xT,
            "xTown": np.ascontiguousarray(xT[:, blk]),
            "w1t": w1t,
            "w2t": w2t,
            "maskT": np.ascontiguousarray(maskT[:, blk]),
            "tdup": td, "sdup": sd, "dmult": dm, "sonehot": soh,
        })
    return in_maps


def declare_io(nc, cfg: GATConfig):
    n, s = cfg.n, cfg.s
    dup_cap = cfg.dup_rounds * P
    io = {
        "xT": nc.dram_tensor("xT", [cfg.f_in, n], F32R, kind="ExternalInput").ap(),
        "xTown": nc.dram_tensor("xTown", [cfg.f_in, s], F32R, kind="ExternalInput").ap(),
        "w1t": nc.dram_tensor("w1t", [cfg.f_in, cfg.w1cols], F32R, kind="ExternalInput").ap(),
        "w2t": nc.dram_tensor("w2t", [cfg.fcat, cfg.w2cols], F32R, kind="ExternalInput").ap(),
        "maskT": nc.dram_tensor("maskT", [n, s], FP8, kind="ExternalInput").ap(),
        "tdup": nc.dram_tensor("tdup", [dup_cap, 1], I32, kind="ExternalInput").ap(),
        "sdup": nc.dram_tensor("sdup", [dup_cap, 1], I32, kind="ExternalInput").ap(),
        "dmult": nc.dram_tensor("dmult", [dup_cap, 1], F32, kind="ExternalInput").ap(),
        "sonehot": nc.dram_tensor("sonehot", [dup_cap, s], F32R, kind="ExternalInput").ap(),
        "out": nc.dram_tensor("out", [s, cfg.nclass], F32, kind="ExternalOutput").ap(),
    }
    return io


def build_program(cfg: GATConfig):
    nc = bacc.Bacc("TRN2", target_bir_lowering=False, debug=False,
                   num_devices=cfg.ncores)
    io = declare_io(nc, cfg)
    with tile.TileContext(nc) as tc:
        build_gat_kernel(tc, cfg, io)
    nc.compile()
    return nc


_CACHE = {}


def kernel(x, edge_list, W1, b1, a1, W2, b2, a2, _trace=False, _tmpdir=None):
    cfg = GATConfig()
    assert np.asarray(b1).max() == 0 and np.asarray(b2).max() == 0
    in_maps = preprocess(cfg, np.asarray(x), np.asarray(edge_list),
                         np.asarray(W1), np.asarray(a1),
                         np.asarray(W2), np.asarray(a2))
    if cfg not in _CACHE:
        _CACHE[cfg] = build_program(cfg)
    nc = _CACHE[cfg]
    res = run_bass_kernel_spmd(
        nc, in_maps, core_ids=list(range(cfg.ncores)),
        trace=_trace, tmpdir=_tmpdir,
        **({"trace_cores": [0]} if _trace else {}))
    out = np.concatenate([r["out"] for r in res.results], axis=0)
    kernel._last_results = res
    return out.astype(np.float32)


# revision 14
# speedup vs baseline: 1.5061x; 1.5061x over previous
"""Two-layer GAT on 8 Trainium2 NeuronCores.

Strategy (row-sharded dense attention):
  - Nodes (= rows of the dense NxN attention matrix) are sharded across the 8
    cores; core c owns rows [c*S, (c+1)*S), S = N/8.
  - The adjacency structure is fixed by edge_list, so the 0/1 mask is built on
    the host (fp8, transposed [t, s] layout) and streamed from HBM; it is
    reused by all 4 heads of layer 1 and by layer 2.
  - exp(leaky_relu(as+at)) = max(exp(as+at), exp(0.2*(as+at))): two ACT Exp
    passes with per-partition bias (at columns are produced by the h = x@W.T
    matmul itself, via extra host-precomputed weight columns W.T@a).
  - Attention output is accumulated transposed: outT[f, s] = sum_j
    (h|1)_j.T @ PT_j, with a ones column giving the softmax denominator Z
    for free (softmax without max subtraction - the logit range is small).
  - Duplicate edges (where the reference scatter-add sums e values) are
    excluded from the mask and corrected exactly with a small gather
    (indirect DMA) + rank-limited correction matmul.
  - One small AllGather ([S, 20] per core) carries layer-2 h2 / alpha columns
    between the layers; everything else is local.
"""

import math
from dataclasses import dataclass

import ml_dtypes
import numpy as np

import concourse.bass as bass
import concourse.mybir as mybir
import concourse.tile as tile
from concourse import bacc
from concourse.bass_utils import run_bass_kernel_spmd
from concourse.masks import make_identity

F32 = mybir.dt.float32
F32R = mybir.dt.float32r
BF16 = mybir.dt.bfloat16
FP8 = mybir.dt.float8e4
I32 = mybir.dt.int32
AF = mybir.ActivationFunctionType
OP = mybir.AluOpType
AX = mybir.AxisListType

P = 128


@dataclass(frozen=True)
class GATConfig:
    n: int = 8192          # nodes
    f_in: int = 512        # input features
    nhid: int = 64         # per-head hidden
    heads: int = 4
    nclass: int = 16
    ncores: int = 8
    dup_rounds: int = 1    # ceil(max dup edges per core / 128)

    @property
    def s(self):           # rows per core
        return self.n // self.ncores

    @property
    def jt(self):          # 128-row t tiles
        return self.n // P

    @property
    def sc(self):          # 128-row chunks of own block
        return self.s // P

    @property
    def kt1(self):         # k tiles of f_in
        return self.f_in // P

    @property
    def fcat(self):        # concat feature dim after layer 1
        return self.nhid * self.heads

    @property
    def kt2(self):
        return self.fcat // P

    @property
    def hstride(self):     # per-head column stride in h1buf: h|1|as|at|0.2at
        return self.nhid + 4

    @property
    def w1cols(self):
        return self.hstride * self.heads

    # layer-2 psum/payload columns: h2(nclass) | ones | at | 0.2at | as
    @property
    def w2cols(self):
        return self.nclass + 4


def _ts(i, sz):
    return slice(i * sz, (i + 1) * sz)


def _halves(s):
    return [slice(h0, min(h0 + 512, s)) for h0 in range(0, s, 512)]


def build_gat_kernel(tc, cfg: GATConfig, io):
    """Emit the GAT program. io: dict of DRAM APs (inputs+outputs)."""
    nc = tc.nc
    n, s, jt, sc = cfg.n, cfg.s, cfg.jt, cfg.sc
    hs, nh, nheads = cfg.hstride, cfg.nhid, cfg.heads
    ncl = cfg.nclass
    MCH = 2  # mask j-tiles per DMA chunk
    njc = jt // MCH

    xT, xTown = io["xT"], io["xTown"]
    w1t, w2t, maskT = io["w1t"], io["w2t"], io["maskT"]
    tdup, sdup, dmult, sonehot = io["tdup"], io["sdup"], io["dmult"], io["sonehot"]
    out_dram = io["out"]

    with (
        tc.tile_pool(name="persist", bufs=1) as sb1,
        tc.tile_pool(name="stream", bufs=2) as sb_stream,
        tc.tile_pool(name="etile", bufs=2) as sb_e,
        tc.tile_pool(name="pttile", bufs=2) as sb_pt,
        tc.tile_pool(name="misc", bufs=2) as sb_misc,
        tc.tile_pool(name="ps_small", bufs=2, space="PSUM") as ps_small,
        tc.tile_pool(name="ps_big", bufs=2, space="PSUM") as ps_big,
        tc.tile_pool(name="dram", bufs=1, space="DRAM") as dram,
    ):
        # ---------------- persistent tiles ----------------
        w1t_sb = sb1.tile([P, cfg.kt1, cfg.w1cols], F32R, tag="w1t_sb")
        nc.sync.dma_start(
            w1t_sb[:], w1t[:].rearrange("(k p) c -> p k c", p=P))
        w2t_sb = sb1.tile([P, cfg.kt2, cfg.w2cols], F32R, tag="w2t_sb")
        nc.sync.dma_start(
            w2t_sb[:], w2t[:].rearrange("(k p) c -> p k c", p=P))

        h1buf = sb1.tile([P, jt, cfg.w1cols], F32R, tag="h1buf")
        hl2T = sb1.tile([P, cfg.kt2, s], F32R, tag="hl2T")
        g_all = sb1.tile([P, jt, cfg.w2cols], F32R, tag="g_all")

        ones1 = sb1.tile([1, P], F32R, tag="ones1")
        nc.vector.memset(ones1[:].bitcast(F32), 1.0)
        ident = sb1.tile([P, P], F32, tag="ident")
        make_identity(nc, ident[:])

        # dup inputs resident
        tdup_sb = sb1.tile([P, cfg.dup_rounds, 1], I32, tag="tdup_sb")
        sdup_sb = sb1.tile([P, cfg.dup_rounds, 1], I32, tag="sdup_sb")
        dmult_sb = sb1.tile([P, cfg.dup_rounds, 1], F32, tag="dmult_sb")
        nc.sync.dma_start(tdup_sb[:], tdup[:].rearrange("(r p) c -> p r c", p=P))
        nc.sync.dma_start(sdup_sb[:], sdup[:].rearrange("(r p) c -> p r c", p=P))
        nc.sync.dma_start(dmult_sb[:], dmult[:].rearrange("(r p) c -> p r c", p=P))
        soh_sb = sb1.tile([P, cfg.dup_rounds, s], F32R, tag="soh_sb")
        nc.sync.dma_start(soh_sb[:], sonehot[:].rearrange("(r p) c -> p r c", p=P))

        h1dram = dram.tile([n, cfg.w1cols], F32R)
        payload_dram = dram.tile([s, cfg.w2cols], F32R)
        g_dram = dram.tile([n, cfg.w2cols], F32R)

        # ---------------- phase A: h1 = x @ W1ext (all rows, all heads) ----
        for j in range(jt):
            xtc = sb_stream.tile([P, cfg.kt1, P], F32R, tag="xtc")
            nc.sync.dma_start(
                xtc[:], xT[:, _ts(j, P)].rearrange("(k p) m -> p k m", p=P))
            psA = ps_small.tile([P, cfg.w1cols], F32, tag="small")
            for k in range(cfg.kt1):
                nc.tensor.matmul(
                    psA[:], xtc[:, k, :],
                    w1t_sb[:, k, :],
                    start=(k == 0), stop=(k == cfg.kt1 - 1))
            nc.scalar.copy(h1buf[:, j, :], psA[:])
        # ones columns
        for h in range(nheads):
            nc.vector.memset(h1buf[:, :, h * hs + nh].bitcast(F32), 1.0)
        # h1 -> DRAM for the dup gathers
        nc.sync.dma_start(
            h1dram[:].rearrange("(j p) c -> p j c", p=P), h1buf[:])

        # dup gathers for layer 1 (rows of h1ext at t and s indices)
        hdup1 = []
        sdup1 = []
        for r in range(cfg.dup_rounds):
            ht = sb1.tile([P, cfg.w1cols], F32R, tag=f"hdup1_{r}")
            nc.gpsimd.indirect_dma_start(
                out=ht[:], out_offset=None, in_=h1dram[:],
                in_offset=bass.IndirectOffsetOnAxis(ap=tdup_sb[:, r, :], axis=0))
            hdup1.append(ht)
            hs_ = sb1.tile([P, cfg.w1cols], F32R, tag=f"sdup1_{r}")
            nc.gpsimd.indirect_dma_start(
                out=hs_[:], out_offset=None, in_=h1dram[:],
                in_offset=bass.IndirectOffsetOnAxis(ap=sdup_sb[:, r, :], axis=0))
            sdup1.append(hs_)

        # alpha_src row [1, s] per head
        asrow = []
        for h in range(nheads):
            psr = ps_small.tile([1, s], F32, tag="small")
            for k in range(cfg.kt1):
                xo = sb_stream.tile([P, s], F32R, tag="xo")
                nc.sync.dma_start(xo[:], xTown[_ts(k, P), :])
                for hsl in _halves(s):
                    nc.tensor.matmul(
                        psr[:, hsl],
                        w1t_sb[:, k, h * hs + nh + 1: h * hs + nh + 2],
                        xo[:, hsl],
                        start=(k == 0), stop=(k == cfg.kt1 - 1))
            ar = sb1.tile([1, s], F32R, tag=f"asrow_{h}", name=f"asrow_{h}")
            nc.scalar.copy(ar[:], psr[:])
            asrow.append(ar)

        # ---------------- layer 1 attention, 2 heads per pass -------------
        def attn_pass(heads_list, lhs_getter, at_getter, at2_getter, asb_list,
                      hdup, sdupt, dup_alpha_s_col, dup_alpha_t_col,
                      dup_lhs_cols, psum_tiles, mask_dtype):
            """One full j sweep accumulating outT for the given heads."""
            npass = len(heads_list)
            for jc in range(njc):
                mtile = sb_stream.tile([P, MCH, s], mask_dtype, tag="mtile")
                nc.sync.dma_start(
                    mtile[:],
                    maskT[_ts(jc, MCH * P), :].rearrange(
                        "(c p) ss -> p c ss", p=P))
                for jj in range(MCH):
                    j = jc * MCH + jj
                    for hi, h in enumerate(heads_list):
                        # patched Exp table computes exp(leaky_relu(x))
                        e1 = sb_e.tile([P, s], F32, tag="e1")
                        nc.scalar.activation(
                            e1[:], asb_list[hi][:], AF.Exp,
                            bias=at_getter(j, h), scale=1.0)
                        pt = sb_pt.tile([P, s], F32R, tag="pt")
                        nc.vector.tensor_mul(pt[:], e1[:], mtile[:, jj, :])
                        for hsl in _halves(s):
                            nc.tensor.matmul(
                                psum_tiles[hi][:, hsl],
                                lhs_getter(j, h),
                                pt[:, hsl],
                                start=(j == 0), stop=False)
            # dup corrections close the accumulation group
            for hi, h in enumerate(heads_list):
                for r in range(cfg.dup_rounds):
                    at_d = hdup[r][:, dup_alpha_t_col(h):dup_alpha_t_col(h) + 1]
                    as_d = sdupt[r][:, dup_alpha_s_col(h):dup_alpha_s_col(h) + 1]
                    u = sb_misc.tile([P, 1], F32, tag="u_d")
                    nc.vector.tensor_add(u[:], at_d, as_d)
                    # w8 = 0.8*relu(u)
                    w8 = sb_misc.tile([P, 1], F32, tag="w8_d")
                    nc.vector.tensor_scalar(
                        out=w8[:], in0=u[:], scalar1=0.0, scalar2=0.8,
                        op0=OP.max, op1=OP.mult)
                    # w = (0.2*u + w8) * mult
                    w_ = sb_misc.tile([P, 1], F32, tag="w_d")
                    nc.vector.scalar_tensor_tensor(
                        out=w_[:], in0=u[:], scalar=0.2, in1=w8[:],
                        op0=OP.mult, op1=OP.add)
                    nc.vector.tensor_mul(w_[:], w_[:], dmult_sb[:, r, :])
                    # patched exp: split w into pos/neg parts (neg scaled by 5)
                    wp = sb_misc.tile([P, 1], F32, tag="wp_d")
                    nc.vector.tensor_single_scalar(wp[:], w_[:], 0.0, OP.max)
                    wn = sb_misc.tile([P, 1], F32, tag="wn_d")
                    nc.vector.tensor_single_scalar(wn[:], w_[:], 0.0, OP.min)
                    dp = sb_misc.tile([P, 1], F32, tag="dp_d")
                    nc.scalar.activation(dp[:], wp[:], AF.Exp)
                    dn = sb_misc.tile([P, 1], F32, tag="dn_d")
                    nc.scalar.activation(dn[:], wn[:], AF.Exp, scale=5.0)
                    dl = sb_misc.tile([P, 1], F32, tag="dl_d")
                    nc.vector.tensor_mul(dl[:], dp[:], dn[:])
                    c0, c1 = dup_lhs_cols(h)
                    hsc = sb_misc.tile([P, c1 - c0], F32R, tag="hsc_d")
                    nc.vector.tensor_scalar_mul(hsc[:], hdup[r][:, c0:c1], dl[:])
                    for hsl in _halves(s):
                        nc.tensor.matmul(
                            psum_tiles[hi][:, hsl],
                            hsc[:],
                            soh_sb[:, r, hsl],
                            start=False, stop=(r == cfg.dup_rounds - 1))

        def build_asb(ar):
            """[1,s] row AP -> [128,s] broadcast tile (PE outer product)."""
            psb = ps_big.tile([P, s], F32, tag="big")
            for hsl in _halves(s):
                nc.tensor.matmul(psb[:, hsl], ones1[:], ar[:, hsl],
                                 start=True, stop=True)
            asb = sb_misc.tile([P, s], F32, tag="asb")
            nc.scalar.copy(asb[:], psb[:])
            return asb

        for pp in range(nheads // 2):
            hpair = [2 * pp, 2 * pp + 1]
            asb_list = [build_asb(asrow[h][0:1, :]) for h in hpair]
            psum_tiles = [
                ps_big.tile([nh + 1, s], F32, tag="big", name=f"attnps_{pp}_{i}")
                for i in range(2)]
            attn_pass(
                hpair,
                lhs_getter=lambda j, h: h1buf[:, j, h * hs: h * hs + nh + 1],
                at_getter=lambda j, h: h1buf[:, j, h * hs + nh + 2: h * hs + nh + 3],
                at2_getter=lambda j, h: h1buf[:, j, h * hs + nh + 3: h * hs + nh + 4],
                asb_list=asb_list,
                hdup=hdup1, sdupt=sdup1,
                dup_alpha_s_col=lambda h: h * hs + nh + 1,
                dup_alpha_t_col=lambda h: h * hs + nh + 2,
                dup_lhs_cols=lambda h: (h * hs, h * hs + nh + 1),
                psum_tiles=psum_tiles, mask_dtype=FP8)

            # normalize + ELU into hl2T (transposed feature-major layout)
            for hi, h in enumerate(hpair):
                otT = sb_misc.tile([nh + 1, s], F32, tag="otT")
                nc.scalar.copy(otT[:], psum_tiles[hi][:])
                rz = sb_misc.tile([1, s], F32R, tag="rz", bufs=1)
                with nc.allow_low_precision(reason="fp22 recip row for PE bcast"):
                    nc.vector.reciprocal(rz[:], otT[nh:nh + 1, :])
                rzb = ps_big.tile([nh, s], F32, tag="big")
                for hsl in _halves(s):
                    nc.tensor.matmul(rzb[:, hsl], ones1[:, :nh],
                                     rz[0:1, hsl], start=True, stop=True)
                xn = sb_misc.tile([nh, s], F32, tag="xn", bufs=1)
                nc.vector.tensor_mul(xn[:], otT[:nh, :], rzb[:])
                t1 = sb_misc.tile([nh, s], F32, tag="t1", bufs=1)
                nc.vector.tensor_single_scalar(t1[:], xn[:], 0.0, OP.min)
                t2 = sb_misc.tile([nh, s], F32, tag="t2", bufs=1)
                nc.scalar.activation(t2[:], t1[:], AF.Exp, scale=5.0)
                t4 = sb_misc.tile([nh, s], F32, tag="t4", bufs=1)
                nc.vector.tensor_scalar(
                    out=t4[:], in0=xn[:], scalar1=0.0, scalar2=-1.0,
                    op0=OP.max, op1=OP.add)
                kt = h // 2
                ro = nh * (h % 2)
                nc.vector.tensor_add(
                    hl2T[ro:ro + nh, kt, :], t4[:], t2[:])

        # ---------------- layer 2 build + AllGather ------------------------
        a2srow = sb1.tile([1, s], F32R, tag="a2srow")
        for c in range(sc):
            ps2 = ps_small.tile([P, cfg.w2cols], F32, tag="small")
            for k in range(cfg.kt2):
                nc.tensor.matmul(
                    ps2[:], hl2T[:, k, _ts(c, P)],
                    w2t_sb[:, k, :],
                    start=(k == 0), stop=(k == cfg.kt2 - 1))
            pay = sb_misc.tile([P, cfg.w2cols], F32R, tag="pay")
            nc.scalar.copy(pay[:], ps2[:])
            nc.vector.memset(pay[:, ncl:ncl + 1].bitcast(F32), 1.0)
            nc.sync.dma_start(payload_dram[_ts(c, P), :], pay[:])
            # alpha2_src row segment (col ncl+3)
            nc.sync.dma_start(
                a2srow[0:1, _ts(c, P)], pay[:, ncl + 3:ncl + 4])

        nc.gpsimd.collective_compute(
            "AllGather", OP.bypass,
            replica_groups=[list(range(cfg.ncores))],
            ins=[payload_dram.opt()], outs=[g_dram.opt()])

        nc.sync.dma_start(
            g_all[:], g_dram[:].rearrange("(j p) c -> p j c", p=P))

        hdup2 = []
        sdup2 = []
        for r in range(cfg.dup_rounds):
            ht = sb1.tile([P, cfg.w2cols], F32R, tag=f"hdup2_{r}")
            nc.gpsimd.indirect_dma_start(
                out=ht[:], out_offset=None, in_=g_dram[:],
                in_offset=bass.IndirectOffsetOnAxis(ap=tdup_sb[:, r, :], axis=0))
            hdup2.append(ht)
            hs_ = sb1.tile([P, cfg.w2cols], F32R, tag=f"sdup2_{r}")
            nc.gpsimd.indirect_dma_start(
                out=hs_[:], out_offset=None, in_=g_dram[:],
                in_offset=bass.IndirectOffsetOnAxis(ap=sdup_sb[:, r, :], axis=0))
            sdup2.append(hs_)

        # ---------------- layer 2 attention --------------------------------
        asb2 = [build_asb(a2srow[0:1, :])]
        psum2 = [ps_big.tile([ncl + 1, s], F32, tag="big", name="attnps2")]
        attn_pass(
            [0],
            lhs_getter=lambda j, h: g_all[:, j, 0:ncl + 1],
            at_getter=lambda j, h: g_all[:, j, ncl + 1:ncl + 2],
            at2_getter=lambda j, h: g_all[:, j, ncl + 2:ncl + 3],
            asb_list=asb2,
            hdup=hdup2, sdupt=sdup2,
            dup_alpha_s_col=lambda h: ncl + 3,
            dup_alpha_t_col=lambda h: ncl + 1,
            dup_lhs_cols=lambda h: (0, ncl + 1),
            psum_tiles=psum2, mask_dtype=FP8)

        # ---------------- epilogue: transpose, normalize, log_softmax ------
        otT2 = sb1.tile([ncl + 1, s], F32, tag="otT2")
        nc.scalar.copy(otT2[:], psum2[0][:])
        for c in range(sc):
            pst = ps_small.tile([P, ncl + 1], F32, tag="small")
            nc.tensor.transpose(
                pst[:], otT2[:, _ts(c, P)], ident[0:ncl + 1, 0:ncl + 1])
            rz = sb_misc.tile([P, 1], F32, tag="rz2")
            nc.vector.reciprocal(rz[:], pst[:, ncl:ncl + 1])
            lg = sb_misc.tile([P, ncl], F32, tag="lg")
            nc.vector.tensor_scalar_mul(lg[:], pst[:, 0:ncl], rz[:])
            m = sb_misc.tile([P, 1], F32, tag="m2")
            nc.vector.tensor_reduce(m[:], lg[:], axis=AX.X, op=OP.max)
            negm = sb_misc.tile([P, 1], F32, tag="negm")
            nc.vector.tensor_single_scalar(negm[:], m[:], -5.0, OP.mult)
            exd = sb_misc.tile([P, ncl], F32, tag="exd")
            zs = sb_misc.tile([P, 1], F32, tag="zs")
            nc.scalar.activation(exd[:], lg[:], AF.Exp, scale=5.0,
                                 bias=negm[:, 0:1], accum_out=zs[:, 0:1])
            lnz = sb_misc.tile([P, 1], F32, tag="lnz")
            nc.scalar.activation(lnz[:], zs[:], AF.Ln)
            fin = sb_misc.tile([P, ncl], F32, tag="fin")
            nc.vector.tensor_scalar(
                out=fin[:], in0=lg[:], scalar1=m[:, 0:1], scalar2=lnz[:, 0:1],
                op0=OP.subtract, op1=OP.subtract)
            nc.sync.dma_start(out_dram[_ts(c, P), :], fin[:])


# ======================= host side =======================================


def _leaky(x, alpha=0.2):
    return np.where(x > 0, x, alpha * x)


def preprocess(cfg: GATConfig, x, edge_list, W1, a1, W2, a2):
    """Build per-core input maps (numpy only)."""
    n, s = cfg.n, cfg.s
    src = np.asarray(edge_list[0]).astype(np.int64)
    tgt = np.asarray(edge_list[1]).astype(np.int64)
    key = src * n + tgt
    uniq, counts = np.unique(key, return_counts=True)
    us = (uniq // n).astype(np.int32)   # row (softmax) index
    ut = (uniq % n).astype(np.int32)    # col index
    singles = counts == 1
    dups = ~singles

    # transposed mask [t, s]; dup cells excluded (corrected exactly later)
    maskT = np.zeros((n, n), dtype=ml_dtypes.float8_e4m3)
    maskT[ut[singles], us[singles]] = 1.0

    # row coverage check: every row must have at least one edge
    row_deg = np.bincount(us, minlength=n)
    assert row_deg.min() > 0, "empty adjacency row: kernel assumes none"

    d_s, d_t, d_m = us[dups], ut[dups], counts[dups].astype(np.float32)

    x = np.asarray(x, dtype=np.float32)
    xT = np.ascontiguousarray(x.T)

    W1 = np.asarray(W1, dtype=np.float32)
    a1 = np.asarray(a1, dtype=np.float32)
    W2 = np.asarray(W2, dtype=np.float32)
    a2 = np.asarray(a2, dtype=np.float32)

    hs, nh = cfg.hstride, cfg.nhid
    w1t = np.zeros((cfg.f_in, cfg.w1cols), np.float32)
    for h in range(cfg.heads):
        w1t[:, h * hs: h * hs + nh] = W1[h].T
        va_s = W1[h].T @ a1[h, :nh]
        va_t = W1[h].T @ a1[h, nh:]
        w1t[:, h * hs + nh + 1] = va_s
        w1t[:, h * hs + nh + 2] = va_t
        w1t[:, h * hs + nh + 3] = 0.2 * va_t

    ncl = cfg.nclass
    w2t = np.zeros((cfg.fcat, cfg.w2cols), np.float32)
    w2t[:, 0:ncl] = W2.T
    va2_s = W2.T @ a2[:ncl]
    va2_t = W2.T @ a2[ncl:]
    w2t[:, ncl + 1] = va2_t
    w2t[:, ncl + 2] = 0.2 * va2_t
    w2t[:, ncl + 3] = va2_s

    in_maps = []
    max_dups = 0
    for c in range(cfg.ncores):
        blk = slice(c * s, (c + 1) * s)
        mine = (d_s >= c * s) & (d_s < (c + 1) * s)
        max_dups = max(max_dups, int(mine.sum()))
    dup_cap = cfg.dup_rounds * P
    assert max_dups <= dup_cap, f"{max_dups} dups > cap {dup_cap}"

    for c in range(cfg.ncores):
        blk = slice(c * s, (c + 1) * s)
        mine = np.nonzero((d_s >= c * s) & (d_s < (c + 1) * s))[0]
        k = len(mine)
        td = np.zeros((dup_cap, 1), np.int32)
        sd = np.zeros((dup_cap, 1), np.int32)
        dm = np.zeros((dup_cap, 1), np.float32)
        soh = np.zeros((dup_cap, s), np.float32)
        td[:k, 0] = d_t[mine]
        sd[:k, 0] = d_s[mine]
        dm[:k, 0] = d_m[mine]
        soh[np.arange(k), d_s[mine] - c * s] = 1.0
        in_maps.append({
            "xT": xT,
            "xTown": np.ascontiguousarray(xT[:, blk]),
            "w1t": w1t,
            "w2t": w2t,
            "maskT": np.ascontiguousarray(maskT[:, blk]),
            "tdup": td, "sdup": sd, "dmult": dm, "sonehot": soh,
        })
    return in_maps


def declare_io(nc, cfg: GATConfig):
    n, s = cfg.n, cfg.s
    dup_cap = cfg.dup_rounds * P
    io = {
        "xT": nc.dram_tensor("xT", [cfg.f_in, n], F32R, kind="ExternalInput").ap(),
        "xTown": nc.dram_tensor("xTown", [cfg.f_in, s], F32R, kind="ExternalInput").ap(),
        "w1t": nc.dram_tensor("w1t", [cfg.f_in, cfg.w1cols], F32R, kind="ExternalInput").ap(),
        "w2t": nc.dram_tensor("w2t", [cfg.fcat, cfg.w2cols], F32R, kind="ExternalInput").ap(),
        "maskT": nc.dram_tensor("maskT", [n, s], FP8, kind="ExternalInput").ap(),
        "tdup": nc.dram_tensor("tdup", [dup_cap, 1], I32, kind="ExternalInput").ap(),
        "sdup": nc.dram_tensor("sdup", [dup_cap, 1], I32, kind="ExternalInput").ap(),
        "dmult": nc.dram_tensor("dmult", [dup_cap, 1], F32, kind="ExternalInput").ap(),
        "sonehot": nc.dram_tensor("sonehot", [dup_cap, s], F32R, kind="ExternalInput").ap(),
        "out": nc.dram_tensor("out", [s, cfg.nclass], F32, kind="ExternalOutput").ap(),
    }
    return io


_ACT_PATCH_DIR = None


def install_patched_act_tables():
    """Repoint BASS_ACT_ROOT_JSON_PATH at a copy of the stock PWP tables in
    which exp's negative-domain buckets compute exp(0.2*x) instead of exp(x).
    The Exp activation then evaluates exp(leaky_relu(x)) in a single pass.
    (All other exp uses in this kernel feed arguments <= 0 scaled by 5, or
    split pos/neg, so they still compute a true exp.)"""
    global _ACT_PATCH_DIR
    import json
    import os
    import shutil
    import tempfile

    if _ACT_PATCH_DIR is not None:
        os.environ["BASS_ACT_ROOT_JSON_PATH"] = os.path.join(
            _ACT_PATCH_DIR, "act_info.json")
        return

    from neuronxcc.driver.Job import Job
    from neuronxcc.driver.jobs.support.FindActInfo import findActInfoFile

    src_json = findActInfoFile(Job.getPackageDir(), "gen3")
    src_dir = os.path.dirname(src_json)
    pwp_jsons = os.path.join(os.path.dirname(src_dir), "pwp_jsons")

    dst = tempfile.mkdtemp(prefix="act_lrelu_")
    for f in os.listdir(src_dir):
        shutil.copy(os.path.join(src_dir, f), os.path.join(dst, f))

    exp_def = json.load(open(os.path.join(pwp_jsons, "exp_400p.json")))
    neg_secs = []
    for e in exp_def["neg_exponents"]:
        for sct in e["exponent_sections"]:
            neg_secs.append(np.array(
                [sct["d0"]["int"], sct["d1"]["int"], sct["d2"]["int"],
                 sct["d3"]["int"], sct["x"]["int"]], dtype=np.uint32))

    info = json.load(open(os.path.join(dst, "act_info.json")))
    for st in info["act_func_sets"]:
        if "exp" not in st["act"]:
            continue
        path = os.path.join(dst, st["bkt_bin"])
        bkt = np.fromfile(path, dtype=np.uint32)
        view = np.lib.stride_tricks.sliding_window_view(bkt, 5)
        n_patched = 0
        for sec in neg_secs:
            m = np.where(np.all(view == sec, axis=1))[0]
            if len(m) != 1:
                continue
            i = int(m[0])
            x0 = float(sec[4:5].view(np.float32)[0])
            f = np.float32(math.exp(0.2 * x0))
            coef = np.array([f, 0.2 * f, 0.02 * f, (0.2 ** 3 / 6.0) * f],
                            dtype=np.float32)
            bkt[i:i + 4] = coef.view(np.uint32)
            n_patched += 1
        assert n_patched == len(neg_secs), (st["name"], n_patched)
        bkt.tofile(path)

    _ACT_PATCH_DIR = dst
    os.environ["BASS_ACT_ROOT_JSON_PATH"] = os.path.join(dst, "act_info.json")


def build_program(cfg: GATConfig):
    nc = bacc.Bacc("TRN2", target_bir_lowering=False, debug=False,
                   num_devices=cfg.ncores)
    io = declare_io(nc, cfg)
    with tile.TileContext(nc) as tc:
        build_gat_kernel(tc, cfg, io)
    nc.compile()
    return nc


_CACHE = {}


def kernel(x, edge_list, W1, b1, a1, W2, b2, a2, _trace=False, _tmpdir=None):
    cfg = GATConfig()
    assert np.asarray(b1).max() == 0 and np.asarray(b2).max() == 0
    in_maps = preprocess(cfg, np.asarray(x), np.asarray(edge_list),
                         np.asarray(W1), np.asarray(a1),
                         np.asarray(W2), np.asarray(a2))
    install_patched_act_tables()
    if cfg not in _CACHE:
        _CACHE[cfg] = build_program(cfg)
    nc = _CACHE[cfg]
    res = run_bass_kernel_spmd(
        nc, in_maps, core_ids=list(range(cfg.ncores)),
        trace=_trace, tmpdir=_tmpdir,
        **({"trace_cores": [0]} if _trace else {}))
    out = np.concatenate([r["out"] for r in res.results], axis=0)
    kernel._last_results = res
    return out.astype(np.float32)


# revision 16
# speedup vs baseline: 1.5338x; 1.0184x over previous
"""Two-layer GAT on 8 Trainium2 NeuronCores.

Strategy (row-sharded dense attention):
  - Nodes (= rows of the dense NxN attention matrix) are sharded across the 8
    cores; core c owns rows [c*S, (c+1)*S), S = N/8.
  - The adjacency structure is fixed by edge_list, so the 0/1 mask is built on
    the host (fp8, transposed [t, s] layout) and streamed from HBM; it is
    reused by all 4 heads of layer 1 and by layer 2.
  - exp(leaky_relu(as+at)) = max(exp(as+at), exp(0.2*(as+at))): two ACT Exp
    passes with per-partition bias (at columns are produced by the h = x@W.T
    matmul itself, via extra host-precomputed weight columns W.T@a).
  - Attention output is accumulated transposed: outT[f, s] = sum_j
    (h|1)_j.T @ PT_j, with a ones column giving the softmax denominator Z
    for free (softmax without max subtraction - the logit range is small).
  - Duplicate edges (where the reference scatter-add sums e values) are
    excluded from the mask and corrected exactly with a small gather
    (indirect DMA) + rank-limited correction matmul.
  - One small AllGather ([S, 20] per core) carries layer-2 h2 / alpha columns
    between the layers; everything else is local.
"""

import math
from dataclasses import dataclass

import ml_dtypes
import numpy as np

import concourse.bass as bass
import concourse.mybir as mybir
import concourse.tile as tile
from concourse import bacc
from concourse.bass_utils import run_bass_kernel_spmd
from concourse.masks import make_identity

F32 = mybir.dt.float32
F32R = mybir.dt.float32r
BF16 = mybir.dt.bfloat16
FP8 = mybir.dt.float8e4
I32 = mybir.dt.int32
AF = mybir.ActivationFunctionType
OP = mybir.AluOpType
AX = mybir.AxisListType

P = 128


@dataclass(frozen=True)
class GATConfig:
    n: int = 8192          # nodes
    f_in: int = 512        # input features
    nhid: int = 64         # per-head hidden
    heads: int = 4
    nclass: int = 16
    ncores: int = 8
    dup_rounds: int = 1    # ceil(max dup edges per core / 128)

    @property
    def s(self):           # rows per core
        return self.n // self.ncores

    @property
    def jt(self):          # 128-row t tiles
        return self.n // P

    @property
    def sc(self):          # 128-row chunks of own block
        return self.s // P

    @property
    def kt1(self):         # k tiles of f_in
        return self.f_in // P

    @property
    def fcat(self):        # concat feature dim after layer 1
        return self.nhid * self.heads

    @property
    def kt2(self):
        return self.fcat // P

    @property
    def hstride(self):     # per-head column stride in h1buf: h|1|as|at|0.2at
        return self.nhid + 4

    @property
    def w1cols(self):
        return self.hstride * self.heads

    # layer-2 psum/payload columns: h2(nclass) | ones | at | 0.2at | as
    @property
    def w2cols(self):
        return self.nclass + 4


def _ts(i, sz):
    return slice(i * sz, (i + 1) * sz)


def _halves(s):
    return [slice(h0, min(h0 + 512, s)) for h0 in range(0, s, 512)]


def build_gat_kernel(tc, cfg: GATConfig, io):
    """Emit the GAT program. io: dict of DRAM APs (inputs+outputs)."""
    nc = tc.nc
    n, s, jt, sc = cfg.n, cfg.s, cfg.jt, cfg.sc
    hs, nh, nheads = cfg.hstride, cfg.nhid, cfg.heads
    ncl = cfg.nclass
    MCH = 2  # mask j-tiles per DMA chunk
    njc = jt // MCH

    xT, xTown = io["xT"], io["xTown"]
    w1t, w2t, maskT = io["w1t"], io["w2t"], io["maskT"]
    tdup, sdup, dmult, sonehot = io["tdup"], io["sdup"], io["dmult"], io["sonehot"]
    out_dram = io["out"]

    with (
        tc.tile_pool(name="persist", bufs=1) as sb1,
        tc.tile_pool(name="stream", bufs=2) as sb_stream,
        tc.tile_pool(name="etile", bufs=2) as sb_e,
        tc.tile_pool(name="pttile", bufs=2) as sb_pt,
        tc.tile_pool(name="misc", bufs=2) as sb_misc,
        tc.tile_pool(name="ps_small", bufs=2, space="PSUM") as ps_small,
        tc.tile_pool(name="ps_big", bufs=2, space="PSUM") as ps_big,
        tc.tile_pool(name="dram", bufs=1, space="DRAM") as dram,
    ):
        # ---------------- persistent tiles ----------------
        w1t_sb = sb1.tile([P, cfg.kt1, cfg.w1cols], F32R, tag="w1t_sb")
        nc.sync.dma_start(
            w1t_sb[:], w1t[:].rearrange("(k p) c -> p k c", p=P))
        w2t_sb = sb1.tile([P, cfg.kt2, cfg.w2cols], F32R, tag="w2t_sb")
        nc.sync.dma_start(
            w2t_sb[:], w2t[:].rearrange("(k p) c -> p k c", p=P))

        h1buf = sb1.tile([P, jt, cfg.w1cols], F32R, tag="h1buf")
        hl2T = sb1.tile([P, cfg.kt2, s], F32R, tag="hl2T")
        g_all = sb1.tile([P, jt, cfg.w2cols], F32R, tag="g_all")

        ones1 = sb1.tile([1, P], F32R, tag="ones1")
        nc.vector.memset(ones1[:].bitcast(F32), 1.0)
        ident = sb1.tile([P, P], F32, tag="ident")
        make_identity(nc, ident[:])

        # dup inputs resident
        tdup_sb = sb1.tile([P, cfg.dup_rounds, 1], I32, tag="tdup_sb")
        sdup_sb = sb1.tile([P, cfg.dup_rounds, 1], I32, tag="sdup_sb")
        dmult_sb = sb1.tile([P, cfg.dup_rounds, 1], F32, tag="dmult_sb")
        nc.sync.dma_start(tdup_sb[:], tdup[:].rearrange("(r p) c -> p r c", p=P))
        nc.sync.dma_start(sdup_sb[:], sdup[:].rearrange("(r p) c -> p r c", p=P))
        nc.sync.dma_start(dmult_sb[:], dmult[:].rearrange("(r p) c -> p r c", p=P))
        soh_sb = sb1.tile([P, cfg.dup_rounds, s], F32R, tag="soh_sb")
        nc.sync.dma_start(soh_sb[:], sonehot[:].rearrange("(r p) c -> p r c", p=P))

        h1dram = dram.tile([n, cfg.w1cols], F32R)
        payload_dram = dram.tile([s, cfg.w2cols], F32R)
        g_dram = dram.tile([n, cfg.w2cols], F32R)

        # ---------------- phase A: h1 = x @ W1ext (all rows, all heads) ----
        for j in range(jt):
            xtc = sb_stream.tile([P, cfg.kt1, P], F32R, tag="xtc")
            nc.sync.dma_start(
                xtc[:], xT[:, _ts(j, P)].rearrange("(k p) m -> p k m", p=P))
            psA = ps_small.tile([P, cfg.w1cols], F32, tag="small")
            for k in range(cfg.kt1):
                nc.tensor.matmul(
                    psA[:], xtc[:, k, :],
                    w1t_sb[:, k, :],
                    start=(k == 0), stop=(k == cfg.kt1 - 1))
            nc.scalar.copy(h1buf[:, j, :], psA[:])
        # ones columns
        for h in range(nheads):
            nc.vector.memset(h1buf[:, :, h * hs + nh].bitcast(F32), 1.0)
        # h1 -> DRAM for the dup gathers
        nc.sync.dma_start(
            h1dram[:].rearrange("(j p) c -> p j c", p=P), h1buf[:])

        # dup gathers for layer 1 (rows of h1ext at t and s indices)
        hdup1 = []
        sdup1 = []
        for r in range(cfg.dup_rounds):
            ht = sb1.tile([P, cfg.w1cols], F32R, tag=f"hdup1_{r}")
            nc.gpsimd.indirect_dma_start(
                out=ht[:], out_offset=None, in_=h1dram[:],
                in_offset=bass.IndirectOffsetOnAxis(ap=tdup_sb[:, r, :], axis=0))
            hdup1.append(ht)
            hs_ = sb1.tile([P, cfg.w1cols], F32R, tag=f"sdup1_{r}")
            nc.gpsimd.indirect_dma_start(
                out=hs_[:], out_offset=None, in_=h1dram[:],
                in_offset=bass.IndirectOffsetOnAxis(ap=sdup_sb[:, r, :], axis=0))
            sdup1.append(hs_)

        # alpha_src row [1, s] per head
        asrow = []
        for h in range(nheads):
            psr = ps_small.tile([1, s], F32, tag="small")
            for k in range(cfg.kt1):
                xo = sb_stream.tile([P, s], F32R, tag="xo")
                nc.sync.dma_start(xo[:], xTown[_ts(k, P), :])
                for hsl in _halves(s):
                    nc.tensor.matmul(
                        psr[:, hsl],
                        w1t_sb[:, k, h * hs + nh + 1: h * hs + nh + 2],
                        xo[:, hsl],
                        start=(k == 0), stop=(k == cfg.kt1 - 1))
            ar = sb1.tile([1, s], F32R, tag=f"asrow_{h}", name=f"asrow_{h}")
            nc.scalar.copy(ar[:], psr[:])
            asrow.append(ar)

        # ---------------- layer 1 attention, 2 heads per pass -------------
        def attn_pass(heads_list, lhs_getter, at_getter, at2_getter, asb_list,
                      hdup, sdupt, dup_alpha_s_col, dup_alpha_t_col,
                      dup_lhs_cols, psum_tiles, mask_dtype):
            """One full j sweep accumulating outT for the given heads."""
            npass = len(heads_list)
            for jc in range(njc):
                # mask chunk duplicated once per pass-head so the mask
                # multiply for both heads is a single contiguous TT
                mtile = sb_stream.tile([P, MCH, npass, s], mask_dtype,
                                       tag="mtile")
                for cp in range(npass):
                    nc.sync.dma_start(
                        mtile[:, :, cp, :],
                        maskT[_ts(jc, MCH * P), :].rearrange(
                            "(c p) ss -> p c ss", p=P))
                for jj in range(MCH):
                    j = jc * MCH + jj
                    # patched Exp table computes exp(leaky_relu(x))
                    epair = sb_e.tile([P, npass, s], F32, tag="e1")
                    for hi, h in enumerate(heads_list):
                        nc.scalar.activation(
                            epair[:, hi, :], asb_list[hi][:], AF.Exp,
                            bias=at_getter(j, h), scale=1.0)
                    ptp = sb_pt.tile([P, npass, s], F32R, tag="pt")
                    nc.vector.tensor_mul(ptp[:], epair[:], mtile[:, jj, :, :])
                    for hi, h in enumerate(heads_list):
                        for hsl in _halves(s):
                            nc.tensor.matmul(
                                psum_tiles[hi][:, hsl],
                                lhs_getter(j, h),
                                ptp[:, hi, hsl],
                                start=(j == 0), stop=False)
            # dup corrections close the accumulation group
            for hi, h in enumerate(heads_list):
                for r in range(cfg.dup_rounds):
                    at_d = hdup[r][:, dup_alpha_t_col(h):dup_alpha_t_col(h) + 1]
                    as_d = sdupt[r][:, dup_alpha_s_col(h):dup_alpha_s_col(h) + 1]
                    u = sb_misc.tile([P, 1], F32, tag="u_d")
                    nc.vector.tensor_add(u[:], at_d, as_d)
                    # w8 = 0.8*relu(u)
                    w8 = sb_misc.tile([P, 1], F32, tag="w8_d")
                    nc.vector.tensor_scalar(
                        out=w8[:], in0=u[:], scalar1=0.0, scalar2=0.8,
                        op0=OP.max, op1=OP.mult)
                    # w = (0.2*u + w8) * mult
                    w_ = sb_misc.tile([P, 1], F32, tag="w_d")
                    nc.vector.scalar_tensor_tensor(
                        out=w_[:], in0=u[:], scalar=0.2, in1=w8[:],
                        op0=OP.mult, op1=OP.add)
                    nc.vector.tensor_mul(w_[:], w_[:], dmult_sb[:, r, :])
                    # patched exp: split w into pos/neg parts (neg scaled by 5)
                    wp = sb_misc.tile([P, 1], F32, tag="wp_d")
                    nc.vector.tensor_single_scalar(wp[:], w_[:], 0.0, OP.max)
                    wn = sb_misc.tile([P, 1], F32, tag="wn_d")
                    nc.vector.tensor_single_scalar(wn[:], w_[:], 0.0, OP.min)
                    dp = sb_misc.tile([P, 1], F32, tag="dp_d")
                    nc.scalar.activation(dp[:], wp[:], AF.Exp)
                    dn = sb_misc.tile([P, 1], F32, tag="dn_d")
                    nc.scalar.activation(dn[:], wn[:], AF.Exp, scale=5.0)
                    dl = sb_misc.tile([P, 1], F32, tag="dl_d")
                    nc.vector.tensor_mul(dl[:], dp[:], dn[:])
                    c0, c1 = dup_lhs_cols(h)
                    hsc = sb_misc.tile([P, c1 - c0], F32R, tag="hsc_d")
                    nc.vector.tensor_scalar_mul(hsc[:], hdup[r][:, c0:c1], dl[:])
                    for hsl in _halves(s):
                        nc.tensor.matmul(
                            psum_tiles[hi][:, hsl],
                            hsc[:],
                            soh_sb[:, r, hsl],
                            start=False, stop=(r == cfg.dup_rounds - 1))

        def build_asb(ar):
            """[1,s] row AP -> [128,s] broadcast tile (PE outer product)."""
            psb = ps_big.tile([P, s], F32, tag="big")
            for hsl in _halves(s):
                nc.tensor.matmul(psb[:, hsl], ones1[:], ar[:, hsl],
                                 start=True, stop=True)
            asb = sb_misc.tile([P, s], F32, tag="asb")
            nc.scalar.copy(asb[:], psb[:])
            return asb

        for pp in range(nheads // 2):
            hpair = [2 * pp, 2 * pp + 1]
            asb_list = [build_asb(asrow[h][0:1, :]) for h in hpair]
            psum_tiles = [
                ps_big.tile([nh + 1, s], F32, tag="big", name=f"attnps_{pp}_{i}")
                for i in range(2)]
            attn_pass(
                hpair,
                lhs_getter=lambda j, h: h1buf[:, j, h * hs: h * hs + nh + 1],
                at_getter=lambda j, h: h1buf[:, j, h * hs + nh + 2: h * hs + nh + 3],
                at2_getter=lambda j, h: h1buf[:, j, h * hs + nh + 3: h * hs + nh + 4],
                asb_list=asb_list,
                hdup=hdup1, sdupt=sdup1,
                dup_alpha_s_col=lambda h: h * hs + nh + 1,
                dup_alpha_t_col=lambda h: h * hs + nh + 2,
                dup_lhs_cols=lambda h: (h * hs, h * hs + nh + 1),
                psum_tiles=psum_tiles, mask_dtype=FP8)

            # normalize + ELU into hl2T (transposed feature-major layout)
            for hi, h in enumerate(hpair):
                otT = sb_misc.tile([nh + 1, s], F32, tag="otT")
                nc.scalar.copy(otT[:], psum_tiles[hi][:])
                rz = sb_misc.tile([1, s], F32R, tag="rz", bufs=1)
                with nc.allow_low_precision(reason="fp22 recip row for PE bcast"):
                    nc.vector.reciprocal(rz[:], otT[nh:nh + 1, :])
                rzb = ps_big.tile([nh, s], F32, tag="big")
                for hsl in _halves(s):
                    nc.tensor.matmul(rzb[:, hsl], ones1[:, :nh],
                                     rz[0:1, hsl], start=True, stop=True)
                xn = sb_misc.tile([nh, s], F32, tag="elu", bufs=3, name="xn")
                nc.vector.tensor_mul(xn[:], otT[:nh, :], rzb[:])
                t1 = sb_misc.tile([nh, s], F32, tag="elu", bufs=3, name="t1")
                nc.vector.tensor_single_scalar(t1[:], xn[:], 0.0, OP.min)
                t2 = sb_misc.tile([nh, s], F32, tag="elu", bufs=3, name="t2")
                nc.scalar.activation(t2[:], t1[:], AF.Exp, scale=5.0)
                t4 = sb_misc.tile([nh, s], F32, tag="elu", bufs=3, name="t4")
                nc.vector.tensor_scalar(
                    out=t4[:], in0=xn[:], scalar1=0.0, scalar2=-1.0,
                    op0=OP.max, op1=OP.add)
                kt = h // 2
                ro = nh * (h % 2)
                nc.vector.tensor_add(
                    hl2T[ro:ro + nh, kt, :], t4[:], t2[:])

        # ---------------- layer 2 build + AllGather ------------------------
        a2srow = sb1.tile([1, s], F32R, tag="a2srow")
        for c in range(sc):
            ps2 = ps_small.tile([P, cfg.w2cols], F32, tag="small")
            for k in range(cfg.kt2):
                nc.tensor.matmul(
                    ps2[:], hl2T[:, k, _ts(c, P)],
                    w2t_sb[:, k, :],
                    start=(k == 0), stop=(k == cfg.kt2 - 1))
            pay = sb_misc.tile([P, cfg.w2cols], F32R, tag="pay")
            nc.scalar.copy(pay[:], ps2[:])
            nc.vector.memset(pay[:, ncl:ncl + 1].bitcast(F32), 1.0)
            nc.sync.dma_start(payload_dram[_ts(c, P), :], pay[:])
            # alpha2_src row segment (col ncl+3)
            nc.sync.dma_start(
                a2srow[0:1, _ts(c, P)], pay[:, ncl + 3:ncl + 4])

        nc.gpsimd.collective_compute(
            "AllGather", OP.bypass,
            replica_groups=[list(range(cfg.ncores))],
            ins=[payload_dram.opt()], outs=[g_dram.opt()])

        nc.sync.dma_start(
            g_all[:], g_dram[:].rearrange("(j p) c -> p j c", p=P))

        hdup2 = []
        sdup2 = []
        for r in range(cfg.dup_rounds):
            ht = sb1.tile([P, cfg.w2cols], F32R, tag=f"hdup2_{r}")
            nc.gpsimd.indirect_dma_start(
                out=ht[:], out_offset=None, in_=g_dram[:],
                in_offset=bass.IndirectOffsetOnAxis(ap=tdup_sb[:, r, :], axis=0))
            hdup2.append(ht)
            hs_ = sb1.tile([P, cfg.w2cols], F32R, tag=f"sdup2_{r}")
            nc.gpsimd.indirect_dma_start(
                out=hs_[:], out_offset=None, in_=g_dram[:],
                in_offset=bass.IndirectOffsetOnAxis(ap=sdup_sb[:, r, :], axis=0))
            sdup2.append(hs_)

        # ---------------- layer 2 attention --------------------------------
        asb2 = [build_asb(a2srow[0:1, :])]
        psum2 = [ps_big.tile([ncl + 1, s], F32, tag="big", name="attnps2")]
        attn_pass(
            [0],
            lhs_getter=lambda j, h: g_all[:, j, 0:ncl + 1],
            at_getter=lambda j, h: g_all[:, j, ncl + 1:ncl + 2],
            at2_getter=lambda j, h: g_all[:, j, ncl + 2:ncl + 3],
            asb_list=asb2,
            hdup=hdup2, sdupt=sdup2,
            dup_alpha_s_col=lambda h: ncl + 3,
            dup_alpha_t_col=lambda h: ncl + 1,
            dup_lhs_cols=lambda h: (0, ncl + 1),
            psum_tiles=psum2, mask_dtype=FP8)

        # ---------------- epilogue: transpose, normalize, log_softmax ------
        otT2 = sb1.tile([ncl + 1, s], F32, tag="otT2")
        nc.scalar.copy(otT2[:], psum2[0][:])
        for c in range(sc):
            pst = ps_small.tile([P, ncl + 1], F32, tag="small")
            nc.tensor.transpose(
                pst[:], otT2[:, _ts(c, P)], ident[0:ncl + 1, 0:ncl + 1])
            rz = sb_misc.tile([P, 1], F32, tag="rz2")
            nc.vector.reciprocal(rz[:], pst[:, ncl:ncl + 1])
            lg = sb_misc.tile([P, ncl], F32, tag="lg")
            nc.vector.tensor_scalar_mul(lg[:], pst[:, 0:ncl], rz[:])
            m = sb_misc.tile([P, 1], F32, tag="m2")
            nc.vector.tensor_reduce(m[:], lg[:], axis=AX.X, op=OP.max)
            negm = sb_misc.tile([P, 1], F32, tag="negm")
            nc.vector.tensor_single_scalar(negm[:], m[:], -5.0, OP.mult)
            exd = sb_misc.tile([P, ncl], F32, tag="exd")
            zs = sb_misc.tile([P, 1], F32, tag="zs")
            nc.scalar.activation(exd[:], lg[:], AF.Exp, scale=5.0,
                                 bias=negm[:, 0:1], accum_out=zs[:, 0:1])
            lnz = sb_misc.tile([P, 1], F32, tag="lnz")
            nc.scalar.activation(lnz[:], zs[:], AF.Ln)
            fin = sb_misc.tile([P, ncl], F32, tag="fin")
            nc.vector.tensor_scalar(
                out=fin[:], in0=lg[:], scalar1=m[:, 0:1], scalar2=lnz[:, 0:1],
                op0=OP.subtract, op1=OP.subtract)
            nc.sync.dma_start(out_dram[_ts(c, P), :], fin[:])


# ======================= host side =======================================


def _leaky(x, alpha=0.2):
    return np.where(x > 0, x, alpha * x)


def preprocess(cfg: GATConfig, x, edge_list, W1, a1, W2, a2):
    """Build per-core input maps (numpy only)."""
    n, s = cfg.n, cfg.s
    src = np.asarray(edge_list[0]).astype(np.int64)
    tgt = np.asarray(edge_list[1]).astype(np.int64)
    key = src * n + tgt
    uniq, counts = np.unique(key, return_counts=True)
    us = (uniq // n).astype(np.int32)   # row (softmax) index
    ut = (uniq % n).astype(np.int32)    # col index
    singles = counts == 1
    dups = ~singles

    # transposed mask [t, s]; dup cells excluded (corrected exactly later)
    maskT = np.zeros((n, n), dtype=ml_dtypes.float8_e4m3)
    maskT[ut[singles], us[singles]] = 1.0

    # row coverage check: every row must have at least one edge
    row_deg = np.bincount(us, minlength=n)
    assert row_deg.min() > 0, "empty adjacency row: kernel assumes none"

    d_s, d_t, d_m = us[dups], ut[dups], counts[dups].astype(np.float32)

    x = np.asarray(x, dtype=np.float32)
    xT = np.ascontiguousarray(x.T)

    W1 = np.asarray(W1, dtype=np.float32)
    a1 = np.asarray(a1, dtype=np.float32)
    W2 = np.asarray(W2, dtype=np.float32)
    a2 = np.asarray(a2, dtype=np.float32)

    hs, nh = cfg.hstride, cfg.nhid
    w1t = np.zeros((cfg.f_in, cfg.w1cols), np.float32)
    for h in range(cfg.heads):
        w1t[:, h * hs: h * hs + nh] = W1[h].T
        va_s = W1[h].T @ a1[h, :nh]
        va_t = W1[h].T @ a1[h, nh:]
        w1t[:, h * hs + nh + 1] = va_s
        w1t[:, h * hs + nh + 2] = va_t
        w1t[:, h * hs + nh + 3] = 0.2 * va_t

    ncl = cfg.nclass
    w2t = np.zeros((cfg.fcat, cfg.w2cols), np.float32)
    w2t[:, 0:ncl] = W2.T
    va2_s = W2.T @ a2[:ncl]
    va2_t = W2.T @ a2[ncl:]
    w2t[:, ncl + 1] = va2_t
    w2t[:, ncl + 2] = 0.2 * va2_t
    w2t[:, ncl + 3] = va2_s

    in_maps = []
    max_dups = 0
    for c in range(cfg.ncores):
        blk = slice(c * s, (c + 1) * s)
        mine = (d_s >= c * s) & (d_s < (c + 1) * s)
        max_dups = max(max_dups, int(mine.sum()))
    dup_cap = cfg.dup_rounds * P
    assert max_dups <= dup_cap, f"{max_dups} dups > cap {dup_cap}"

    for c in range(cfg.ncores):
        blk = slice(c * s, (c + 1) * s)
        mine = np.nonzero((d_s >= c * s) & (d_s < (c + 1) * s))[0]
        k = len(mine)
        td = np.zeros((dup_cap, 1), np.int32)
        sd = np.zeros((dup_cap, 1), np.int32)
        dm = np.zeros((dup_cap, 1), np.float32)
        soh = np.zeros((dup_cap, s), np.float32)
        td[:k, 0] = d_t[mine]
        sd[:k, 0] = d_s[mine]
        dm[:k, 0] = d_m[mine]
        soh[np.arange(k), d_s[mine] - c * s] = 1.0
        in_maps.append({
            "xT": xT,
            "xTown": np.ascontiguousarray(xT[:, blk]),
            "w1t": w1t,
            "w2t": w2t,
            "maskT": np.ascontiguousarray(maskT[:, blk]),
            "tdup": td, "sdup": sd, "dmult": dm, "sonehot": soh,
        })
    return in_maps


def declare_io(nc, cfg: GATConfig):
    n, s = cfg.n, cfg.s
    dup_cap = cfg.dup_rounds * P
    io = {
        "xT": nc.dram_tensor("xT", [cfg.f_in, n], F32R, kind="ExternalInput").ap(),
        "xTown": nc.dram_tensor("xTown", [cfg.f_in, s], F32R, kind="ExternalInput").ap(),
        "w1t": nc.dram_tensor("w1t", [cfg.f_in, cfg.w1cols], F32R, kind="ExternalInput").ap(),
        "w2t": nc.dram_tensor("w2t", [cfg.fcat, cfg.w2cols], F32R, kind="ExternalInput").ap(),
        "maskT": nc.dram_tensor("maskT", [n, s], FP8, kind="ExternalInput").ap(),
        "tdup": nc.dram_tensor("tdup", [dup_cap, 1], I32, kind="ExternalInput").ap(),
        "sdup": nc.dram_tensor("sdup", [dup_cap, 1], I32, kind="ExternalInput").ap(),
        "dmult": nc.dram_tensor("dmult", [dup_cap, 1], F32, kind="ExternalInput").ap(),
        "sonehot": nc.dram_tensor("sonehot", [dup_cap, s], F32R, kind="ExternalInput").ap(),
        "out": nc.dram_tensor("out", [s, cfg.nclass], F32, kind="ExternalOutput").ap(),
    }
    return io


_ACT_PATCH_DIR = None


def install_patched_act_tables():
    """Repoint BASS_ACT_ROOT_JSON_PATH at a copy of the stock PWP tables in
    which exp's negative-domain buckets compute exp(0.2*x) instead of exp(x).
    The Exp activation then evaluates exp(leaky_relu(x)) in a single pass.
    (All other exp uses in this kernel feed arguments <= 0 scaled by 5, or
    split pos/neg, so they still compute a true exp.)"""
    global _ACT_PATCH_DIR
    import json
    import os
    import shutil
    import tempfile

    if _ACT_PATCH_DIR is not None:
        os.environ["BASS_ACT_ROOT_JSON_PATH"] = os.path.join(
            _ACT_PATCH_DIR, "act_info.json")
        return

    from neuronxcc.driver.Job import Job
    from neuronxcc.driver.jobs.support.FindActInfo import findActInfoFile

    src_json = findActInfoFile(Job.getPackageDir(), "gen3")
    src_dir = os.path.dirname(src_json)
    pwp_jsons = os.path.join(os.path.dirname(src_dir), "pwp_jsons")

    dst = tempfile.mkdtemp(prefix="act_lrelu_")
    for f in os.listdir(src_dir):
        shutil.copy(os.path.join(src_dir, f), os.path.join(dst, f))

    exp_def = json.load(open(os.path.join(pwp_jsons, "exp_400p.json")))
    neg_secs = []
    for e in exp_def["neg_exponents"]:
        for sct in e["exponent_sections"]:
            neg_secs.append(np.array(
                [sct["d0"]["int"], sct["d1"]["int"], sct["d2"]["int"],
                 sct["d3"]["int"], sct["x"]["int"]], dtype=np.uint32))

    info = json.load(open(os.path.join(dst, "act_info.json")))
    for st in info["act_func_sets"]:
        if "exp" not in st["act"]:
            continue
        path = os.path.join(dst, st["bkt_bin"])
        bkt = np.fromfile(path, dtype=np.uint32)
        view = np.lib.stride_tricks.sliding_window_view(bkt, 5)
        n_patched = 0
        for sec in neg_secs:
            m = np.where(np.all(view == sec, axis=1))[0]
            if len(m) != 1:
                continue
            i = int(m[0])
            x0 = float(sec[4:5].view(np.float32)[0])
            f = np.float32(math.exp(0.2 * x0))
            coef = np.array([f, 0.2 * f, 0.02 * f, (0.2 ** 3 / 6.0) * f],
                            dtype=np.float32)
            bkt[i:i + 4] = coef.view(np.uint32)
            n_patched += 1
        assert n_patched == len(neg_secs), (st["name"], n_patched)
        bkt.tofile(path)

    _ACT_PATCH_DIR = dst
    os.environ["BASS_ACT_ROOT_JSON_PATH"] = os.path.join(dst, "act_info.json")


def build_program(cfg: GATConfig):
    nc = bacc.Bacc("TRN2", target_bir_lowering=False, debug=False,
                   num_devices=cfg.ncores)
    io = declare_io(nc, cfg)
    with tile.TileContext(nc) as tc:
        build_gat_kernel(tc, cfg, io)
    nc.compile()
    return nc


_CACHE = {}


def kernel(x, edge_list, W1, b1, a1, W2, b2, a2, _trace=False, _tmpdir=None):
    cfg = GATConfig()
    assert np.asarray(b1).max() == 0 and np.asarray(b2).max() == 0
    in_maps = preprocess(cfg, np.asarray(x), np.asarray(edge_list),
                         np.asarray(W1), np.asarray(a1),
                         np.asarray(W2), np.asarray(a2))
    install_patched_act_tables()
    if cfg not in _CACHE:
        _CACHE[cfg] = build_program(cfg)
    nc = _CACHE[cfg]
    res = run_bass_kernel_spmd(
        nc, in_maps, core_ids=list(range(cfg.ncores)),
        trace=_trace, tmpdir=_tmpdir,
        **({"trace_cores": [0]} if _trace else {}))
    out = np.concatenate([r["out"] for r in res.results], axis=0)
    kernel._last_results = res
    return out.astype(np.float32)


# revision 17
# speedup vs baseline: 1.7692x; 1.1535x over previous
"""Two-layer GAT on 8 Trainium2 NeuronCores.

Strategy (row-sharded dense attention):
  - Nodes (= rows of the dense NxN attention matrix) are sharded across the 8
    cores; core c owns rows [c*S, (c+1)*S), S = N/8.
  - The adjacency structure is fixed by edge_list, so the 0/1 mask is built on
    the host (fp8, transposed [t, s] layout) and streamed from HBM; it is
    reused by all 4 heads of layer 1 and by layer 2.
  - exp(leaky_relu(as+at)) = max(exp(as+at), exp(0.2*(as+at))): two ACT Exp
    passes with per-partition bias (at columns are produced by the h = x@W.T
    matmul itself, via extra host-precomputed weight columns W.T@a).
  - Attention output is accumulated transposed: outT[f, s] = sum_j
    (h|1)_j.T @ PT_j, with a ones column giving the softmax denominator Z
    for free (softmax without max subtraction - the logit range is small).
  - Duplicate edges (where the reference scatter-add sums e values) are
    excluded from the mask and corrected exactly with a small gather
    (indirect DMA) + rank-limited correction matmul.
  - One small AllGather ([S, 20] per core) carries layer-2 h2 / alpha columns
    between the layers; everything else is local.
"""

import math
from dataclasses import dataclass

import ml_dtypes
import numpy as np

import concourse.bass as bass
import concourse.mybir as mybir
import concourse.tile as tile
from concourse import bacc
from concourse.bass_utils import run_bass_kernel_spmd
from concourse.masks import make_identity

F32 = mybir.dt.float32
F32R = mybir.dt.float32r
BF16 = mybir.dt.bfloat16
FP8 = mybir.dt.float8e4
I32 = mybir.dt.int32
AF = mybir.ActivationFunctionType
OP = mybir.AluOpType
AX = mybir.AxisListType

P = 128


@dataclass(frozen=True)
class GATConfig:
    n: int = 8192          # nodes
    f_in: int = 512        # input features
    nhid: int = 64         # per-head hidden
    heads: int = 4
    nclass: int = 16
    ncores: int = 8
    dup_rounds: int = 1    # ceil(max dup edges per core / 128)

    @property
    def s(self):           # rows per core
        return self.n // self.ncores

    @property
    def jt(self):          # 128-row t tiles
        return self.n // P

    @property
    def sc(self):          # 128-row chunks of own block
        return self.s // P

    @property
    def kt1(self):         # k tiles of f_in
        return self.f_in // P

    @property
    def fcat(self):        # concat feature dim after layer 1
        return self.nhid * self.heads

    @property
    def kt2(self):
        return self.fcat // P

    @property
    def hstride(self):     # per-head column stride in h1buf: h|1|as|at|0.2at
        return self.nhid + 4

    @property
    def w1cols(self):
        return self.hstride * self.heads

    # layer-2 psum/payload columns: h2(nclass) | ones | at | 0.2at | as
    @property
    def w2cols(self):
        return self.nclass + 4


def _ts(i, sz):
    return slice(i * sz, (i + 1) * sz)


def _halves(s):
    return [slice(h0, min(h0 + 512, s)) for h0 in range(0, s, 512)]


def build_gat_kernel(tc, cfg: GATConfig, io):
    """Emit the GAT program. io: dict of DRAM APs (inputs+outputs)."""
    nc = tc.nc
    n, s, jt, sc = cfg.n, cfg.s, cfg.jt, cfg.sc
    hs, nh, nheads = cfg.hstride, cfg.nhid, cfg.heads
    ncl = cfg.nclass
    MCH = 2  # mask j-tiles per DMA chunk
    njc = jt // MCH

    xT, xTown = io["xT"], io["xTown"]
    w1t, w2t, maskT = io["w1t"], io["w2t"], io["maskT"]
    tdup, sdup, dmult, sonehot = io["tdup"], io["sdup"], io["dmult"], io["sonehot"]
    out_dram = io["out"]

    with (
        tc.tile_pool(name="persist", bufs=1) as sb1,
        tc.tile_pool(name="stream", bufs=2) as sb_stream,
        tc.tile_pool(name="etile", bufs=2) as sb_e,
        tc.tile_pool(name="pttile", bufs=2) as sb_pt,
        tc.tile_pool(name="misc", bufs=2) as sb_misc,
        tc.tile_pool(name="ps_small", bufs=2, space="PSUM") as ps_small,
        tc.tile_pool(name="ps_big", bufs=2, space="PSUM") as ps_big,
        tc.tile_pool(name="dram", bufs=1, space="DRAM") as dram,
    ):
        # ---------------- persistent tiles ----------------
        w1t_sb = sb1.tile([P, cfg.kt1, cfg.w1cols], F32R, tag="w1t_sb")
        nc.sync.dma_start(
            w1t_sb[:], w1t[:].rearrange("(k p) c -> p k c", p=P))
        w2t_sb = sb1.tile([P, cfg.kt2, cfg.w2cols], F32R, tag="w2t_sb")
        nc.sync.dma_start(
            w2t_sb[:], w2t[:].rearrange("(k p) c -> p k c", p=P))

        h1buf = sb1.tile([P, jt, cfg.w1cols], F32R, tag="h1buf")
        hl2T = sb1.tile([P, cfg.kt2, s], F32R, tag="hl2T")
        g_all = sb1.tile([P, jt, cfg.w2cols], F32R, tag="g_all")

        ones1 = sb1.tile([1, P], F32R, tag="ones1")
        nc.vector.memset(ones1[:].bitcast(F32), 1.0)
        ident = sb1.tile([P, P], F32, tag="ident")
        make_identity(nc, ident[:])

        # dup inputs resident
        tdup_sb = sb1.tile([P, cfg.dup_rounds, 1], I32, tag="tdup_sb")
        sdup_sb = sb1.tile([P, cfg.dup_rounds, 1], I32, tag="sdup_sb")
        dmult_sb = sb1.tile([P, cfg.dup_rounds, 1], F32, tag="dmult_sb")
        nc.sync.dma_start(tdup_sb[:], tdup[:].rearrange("(r p) c -> p r c", p=P))
        nc.sync.dma_start(sdup_sb[:], sdup[:].rearrange("(r p) c -> p r c", p=P))
        nc.sync.dma_start(dmult_sb[:], dmult[:].rearrange("(r p) c -> p r c", p=P))
        soh_sb = sb1.tile([P, cfg.dup_rounds, s], F32R, tag="soh_sb")
        nc.sync.dma_start(soh_sb[:], sonehot[:].rearrange("(r p) c -> p r c", p=P))

        h1dram = dram.tile([n, cfg.w1cols], F32R)
        payload_dram = dram.tile([s, cfg.w2cols], F32R)
        g_dram = dram.tile([n, cfg.w2cols], F32R)

        # ---------------- alpha_src rows (before everything; small) -------
        asrow = []
        for h in range(nheads):
            psr = ps_small.tile([1, s], F32, tag="small")
            for k in range(cfg.kt1):
                xo = sb_stream.tile([P, s], F32R, tag="xo")
                nc.sync.dma_start(xo[:], xTown[_ts(k, P), :])
                for hsl in _halves(s):
                    nc.tensor.matmul(
                        psr[:, hsl],
                        w1t_sb[:, k, h * hs + nh + 1: h * hs + nh + 2],
                        xo[:, hsl],
                        start=(k == 0), stop=(k == cfg.kt1 - 1))
            ar = sb1.tile([1, s], F32R, tag=f"asrow_{h}", name=f"asrow_{h}")
            nc.scalar.copy(ar[:], psr[:])
            asrow.append(ar)

        # ones columns of h1buf, set once; the per-j copies skip these slots
        for h in range(nheads):
            nc.vector.memset(h1buf[:, :, h * hs + nh].bitcast(F32), 1.0)

        def build_h1_j(j):
            """h1[:, j, :] = x_j @ W1ext; interleaved into attention pass 0."""
            xtc = sb_stream.tile([P, cfg.kt1, P], F32R, tag="xtc")
            nc.sync.dma_start(
                xtc[:], xT[:, _ts(j, P)].rearrange("(k p) m -> p k m", p=P))
            psA = ps_small.tile([P, cfg.w1cols], F32, tag="small")
            for k in range(cfg.kt1):
                nc.tensor.matmul(
                    psA[:], xtc[:, k, :],
                    w1t_sb[:, k, :],
                    start=(k == 0), stop=(k == cfg.kt1 - 1))
            # copy h columns and alpha columns, preserving the ones slots
            nc.scalar.copy(
                h1buf[:, j, :].rearrange("p (h c) -> p h c", c=hs)[:, :, 0:nh],
                psA[:].rearrange("p (h c) -> p h c", c=hs)[:, :, 0:nh])
            nc.scalar.copy(
                h1buf[:, j, :].rearrange("p (h c) -> p h c", c=hs)[:, :, nh + 1:hs],
                psA[:].rearrange("p (h c) -> p h c", c=hs)[:, :, nh + 1:hs])

        hdup1 = []
        sdup1 = []

        def after_h1_built():
            """h1 -> DRAM, then the dup gathers (layer-1 corrections)."""
            nc.gpsimd.dma_start(
                h1dram[:].rearrange("(j p) c -> p j c", p=P), h1buf[:])
            for r in range(cfg.dup_rounds):
                ht = sb1.tile([P, cfg.w1cols], F32R, tag=f"hdup1_{r}",
                              name=f"hdup1_{r}")
                nc.gpsimd.indirect_dma_start(
                    out=ht[:], out_offset=None, in_=h1dram[:],
                    in_offset=bass.IndirectOffsetOnAxis(
                        ap=tdup_sb[:, r, :], axis=0))
                hdup1.append(ht)
                hs_ = sb1.tile([P, cfg.w1cols], F32R, tag=f"sdup1_{r}",
                               name=f"sdup1_{r}")
                nc.gpsimd.indirect_dma_start(
                    out=hs_[:], out_offset=None, in_=h1dram[:],
                    in_offset=bass.IndirectOffsetOnAxis(
                        ap=sdup_sb[:, r, :], axis=0))
                sdup1.append(hs_)

        # ---------------- layer 1 attention, 2 heads per pass -------------
        def attn_pass(heads_list, lhs_getter, at_getter, at2_getter, asb_list,
                      hdup, sdupt, dup_alpha_s_col, dup_alpha_t_col,
                      dup_lhs_cols, psum_tiles, mask_dtype,
                      pre_j=None, post_j=None):
            """One full j sweep accumulating outT for the given heads."""
            npass = len(heads_list)
            for jc in range(njc):
                # mask chunk duplicated once per pass-head so the mask
                # multiply for both heads is a single contiguous TT
                mtile = sb_stream.tile([P, MCH, npass, s], mask_dtype,
                                       tag="mtile")
                for cp in range(npass):
                    nc.sync.dma_start(
                        mtile[:, :, cp, :],
                        maskT[_ts(jc, MCH * P), :].rearrange(
                            "(c p) ss -> p c ss", p=P))
                for jj in range(MCH):
                    j = jc * MCH + jj
                    if pre_j is not None:
                        pre_j(j)
                    # patched Exp table computes exp(leaky_relu(x))
                    epair = sb_e.tile([P, npass, s], F32, tag="e1")
                    for hi, h in enumerate(heads_list):
                        nc.scalar.activation(
                            epair[:, hi, :], asb_list[hi][:], AF.Exp,
                            bias=at_getter(j, h), scale=1.0)
                    ptp = sb_pt.tile([P, npass, s], F32R, tag="pt")
                    nc.vector.tensor_mul(ptp[:], epair[:], mtile[:, jj, :, :])
                    for hi, h in enumerate(heads_list):
                        for hsl in _halves(s):
                            nc.tensor.matmul(
                                psum_tiles[hi][:, hsl],
                                lhs_getter(j, h),
                                ptp[:, hi, hsl],
                                start=(j == 0), stop=False)
            if post_j is not None:
                post_j()
            # dup corrections close the accumulation group
            for hi, h in enumerate(heads_list):
                for r in range(cfg.dup_rounds):
                    at_d = hdup[r][:, dup_alpha_t_col(h):dup_alpha_t_col(h) + 1]
                    as_d = sdupt[r][:, dup_alpha_s_col(h):dup_alpha_s_col(h) + 1]
                    u = sb_misc.tile([P, 1], F32, tag="u_d")
                    nc.vector.tensor_add(u[:], at_d, as_d)
                    # w8 = 0.8*relu(u)
                    w8 = sb_misc.tile([P, 1], F32, tag="w8_d")
                    nc.vector.tensor_scalar(
                        out=w8[:], in0=u[:], scalar1=0.0, scalar2=0.8,
                        op0=OP.max, op1=OP.mult)
                    # w = (0.2*u + w8) * mult
                    w_ = sb_misc.tile([P, 1], F32, tag="w_d")
                    nc.vector.scalar_tensor_tensor(
                        out=w_[:], in0=u[:], scalar=0.2, in1=w8[:],
                        op0=OP.mult, op1=OP.add)
                    nc.vector.tensor_mul(w_[:], w_[:], dmult_sb[:, r, :])
                    # patched exp: split w into pos/neg parts (neg scaled by 5)
                    wp = sb_misc.tile([P, 1], F32, tag="wp_d")
                    nc.vector.tensor_single_scalar(wp[:], w_[:], 0.0, OP.max)
                    wn = sb_misc.tile([P, 1], F32, tag="wn_d")
                    nc.vector.tensor_single_scalar(wn[:], w_[:], 0.0, OP.min)
                    dp = sb_misc.tile([P, 1], F32, tag="dp_d")
                    nc.scalar.activation(dp[:], wp[:], AF.Exp)
                    dn = sb_misc.tile([P, 1], F32, tag="dn_d")
                    nc.scalar.activation(dn[:], wn[:], AF.Exp, scale=5.0)
                    dl = sb_misc.tile([P, 1], F32, tag="dl_d")
                    nc.vector.tensor_mul(dl[:], dp[:], dn[:])
                    c0, c1 = dup_lhs_cols(h)
                    hsc = sb_misc.tile([P, c1 - c0], F32R, tag="hsc_d")
                    nc.vector.tensor_scalar_mul(hsc[:], hdup[r][:, c0:c1], dl[:])
                    for hsl in _halves(s):
                        nc.tensor.matmul(
                            psum_tiles[hi][:, hsl],
                            hsc[:],
                            soh_sb[:, r, hsl],
                            start=False, stop=(r == cfg.dup_rounds - 1))

        def build_asb(ar):
            """[1,s] row AP -> [128,s] broadcast tile (PE outer product)."""
            psb = ps_big.tile([P, s], F32, tag="big")
            for hsl in _halves(s):
                nc.tensor.matmul(psb[:, hsl], ones1[:], ar[:, hsl],
                                 start=True, stop=True)
            asb = sb_misc.tile([P, s], F32, tag="asb")
            nc.scalar.copy(asb[:], psb[:])
            return asb

        for pp in range(nheads // 2):
            hpair = [2 * pp, 2 * pp + 1]
            asb_list = [build_asb(asrow[h][0:1, :]) for h in hpair]
            psum_tiles = [
                ps_big.tile([nh + 1, s], F32, tag="big", name=f"attnps_{pp}_{i}")
                for i in range(2)]
            attn_pass(
                hpair,
                lhs_getter=lambda j, h: h1buf[:, j, h * hs: h * hs + nh + 1],
                at_getter=lambda j, h: h1buf[:, j, h * hs + nh + 2: h * hs + nh + 3],
                at2_getter=lambda j, h: h1buf[:, j, h * hs + nh + 3: h * hs + nh + 4],
                asb_list=asb_list,
                hdup=hdup1, sdupt=sdup1,
                dup_alpha_s_col=lambda h: h * hs + nh + 1,
                dup_alpha_t_col=lambda h: h * hs + nh + 2,
                dup_lhs_cols=lambda h: (h * hs, h * hs + nh + 1),
                psum_tiles=psum_tiles, mask_dtype=FP8,
                pre_j=build_h1_j if pp == 0 else None,
                post_j=after_h1_built if pp == 0 else None)

            # normalize + ELU into hl2T (transposed feature-major layout)
            for hi, h in enumerate(hpair):
                otT = sb_misc.tile([nh + 1, s], F32, tag="otT")
                nc.scalar.copy(otT[:], psum_tiles[hi][:])
                rz = sb_misc.tile([1, s], F32R, tag="rz", bufs=1)
                with nc.allow_low_precision(reason="fp22 recip row for PE bcast"):
                    nc.vector.reciprocal(rz[:], otT[nh:nh + 1, :])
                rzb = ps_big.tile([nh, s], F32, tag="big")
                for hsl in _halves(s):
                    nc.tensor.matmul(rzb[:, hsl], ones1[:, :nh],
                                     rz[0:1, hsl], start=True, stop=True)
                xn = sb_misc.tile([nh, s], F32, tag="elu", bufs=3, name="xn")
                nc.vector.tensor_mul(xn[:], otT[:nh, :], rzb[:])
                t1 = sb_misc.tile([nh, s], F32, tag="elu", bufs=3, name="t1")
                nc.vector.tensor_single_scalar(t1[:], xn[:], 0.0, OP.min)
                t2 = sb_misc.tile([nh, s], F32, tag="elu", bufs=3, name="t2")
                nc.scalar.activation(t2[:], t1[:], AF.Exp, scale=5.0)
                t4 = sb_misc.tile([nh, s], F32, tag="elu", bufs=3, name="t4")
                nc.vector.tensor_scalar(
                    out=t4[:], in0=xn[:], scalar1=0.0, scalar2=-1.0,
                    op0=OP.max, op1=OP.add)
                kt = h // 2
                ro = nh * (h % 2)
                nc.vector.tensor_add(
                    hl2T[ro:ro + nh, kt, :], t4[:], t2[:])

        # ---------------- layer 2 build + AllGather ------------------------
        a2srow = sb1.tile([1, s], F32R, tag="a2srow")
        for c in range(sc):
            ps2 = ps_small.tile([P, cfg.w2cols], F32, tag="small")
            for k in range(cfg.kt2):
                nc.tensor.matmul(
                    ps2[:], hl2T[:, k, _ts(c, P)],
                    w2t_sb[:, k, :],
                    start=(k == 0), stop=(k == cfg.kt2 - 1))
            pay = sb_misc.tile([P, cfg.w2cols], F32R, tag="pay")
            nc.scalar.copy(pay[:], ps2[:])
            nc.vector.memset(pay[:, ncl:ncl + 1].bitcast(F32), 1.0)
            nc.sync.dma_start(payload_dram[_ts(c, P), :], pay[:])
            # alpha2_src row segment (col ncl+3)
            nc.sync.dma_start(
                a2srow[0:1, _ts(c, P)], pay[:, ncl + 3:ncl + 4])

        nc.gpsimd.collective_compute(
            "AllGather", OP.bypass,
            replica_groups=[list(range(cfg.ncores))],
            ins=[payload_dram.opt()], outs=[g_dram.opt()])

        nc.sync.dma_start(
            g_all[:], g_dram[:].rearrange("(j p) c -> p j c", p=P))

        hdup2 = []
        sdup2 = []
        for r in range(cfg.dup_rounds):
            ht = sb1.tile([P, cfg.w2cols], F32R, tag=f"hdup2_{r}")
            nc.gpsimd.indirect_dma_start(
                out=ht[:], out_offset=None, in_=g_dram[:],
                in_offset=bass.IndirectOffsetOnAxis(ap=tdup_sb[:, r, :], axis=0))
            hdup2.append(ht)
            hs_ = sb1.tile([P, cfg.w2cols], F32R, tag=f"sdup2_{r}")
            nc.gpsimd.indirect_dma_start(
                out=hs_[:], out_offset=None, in_=g_dram[:],
                in_offset=bass.IndirectOffsetOnAxis(ap=sdup_sb[:, r, :], axis=0))
            sdup2.append(hs_)

        # ---------------- layer 2 attention --------------------------------
        asb2 = [build_asb(a2srow[0:1, :])]
        psum2 = [ps_big.tile([ncl + 1, s], F32, tag="big", name="attnps2")]
        attn_pass(
            [0],
            lhs_getter=lambda j, h: g_all[:, j, 0:ncl + 1],
            at_getter=lambda j, h: g_all[:, j, ncl + 1:ncl + 2],
            at2_getter=lambda j, h: g_all[:, j, ncl + 2:ncl + 3],
            asb_list=asb2,
            hdup=hdup2, sdupt=sdup2,
            dup_alpha_s_col=lambda h: ncl + 3,
            dup_alpha_t_col=lambda h: ncl + 1,
            dup_lhs_cols=lambda h: (0, ncl + 1),
            psum_tiles=psum2, mask_dtype=FP8)

        # ---------------- epilogue: transpose, normalize, log_softmax ------
        otT2 = sb1.tile([ncl + 1, s], F32, tag="otT2")
        nc.scalar.copy(otT2[:], psum2[0][:])
        for c in range(sc):
            pst = ps_small.tile([P, ncl + 1], F32, tag="small")
            nc.tensor.transpose(
                pst[:], otT2[:, _ts(c, P)], ident[0:ncl + 1, 0:ncl + 1])
            rz = sb_misc.tile([P, 1], F32, tag="rz2")
            nc.vector.reciprocal(rz[:], pst[:, ncl:ncl + 1])
            lg = sb_misc.tile([P, ncl], F32, tag="lg")
            nc.vector.tensor_scalar_mul(lg[:], pst[:, 0:ncl], rz[:])
            m = sb_misc.tile([P, 1], F32, tag="m2")
            nc.vector.tensor_reduce(m[:], lg[:], axis=AX.X, op=OP.max)
            negm = sb_misc.tile([P, 1], F32, tag="negm")
            nc.vector.tensor_single_scalar(negm[:], m[:], -5.0, OP.mult)
            exd = sb_misc.tile([P, ncl], F32, tag="exd")
            zs = sb_misc.tile([P, 1], F32, tag="zs")
            nc.scalar.activation(exd[:], lg[:], AF.Exp, scale=5.0,
                                 bias=negm[:, 0:1], accum_out=zs[:, 0:1])
            lnz = sb_misc.tile([P, 1], F32, tag="lnz")
            nc.scalar.activation(lnz[:], zs[:], AF.Ln)
            fin = sb_misc.tile([P, ncl], F32, tag="fin")
            nc.vector.tensor_scalar(
                out=fin[:], in0=lg[:], scalar1=m[:, 0:1], scalar2=lnz[:, 0:1],
                op0=OP.subtract, op1=OP.subtract)
            nc.sync.dma_start(out_dram[_ts(c, P), :], fin[:])


# ======================= host side =======================================


def _leaky(x, alpha=0.2):
    return np.where(x > 0, x, alpha * x)


def preprocess(cfg: GATConfig, x, edge_list, W1, a1, W2, a2):
    """Build per-core input maps (numpy only)."""
    n, s = cfg.n, cfg.s
    src = np.asarray(edge_list[0]).astype(np.int64)
    tgt = np.asarray(edge_list[1]).astype(np.int64)
    key = src * n + tgt
    uniq, counts = np.unique(key, return_counts=True)
    us = (uniq // n).astype(np.int32)   # row (softmax) index
    ut = (uniq % n).astype(np.int32)    # col index
    singles = counts == 1
    dups = ~singles

    # transposed mask [t, s]; dup cells excluded (corrected exactly later)
    maskT = np.zeros((n, n), dtype=ml_dtypes.float8_e4m3)
    maskT[ut[singles], us[singles]] = 1.0

    # row coverage check: every row must have at least one edge
    row_deg = np.bincount(us, minlength=n)
    assert row_deg.min() > 0, "empty adjacency row: kernel assumes none"

    d_s, d_t, d_m = us[dups], ut[dups], counts[dups].astype(np.float32)

    x = np.asarray(x, dtype=np.float32)
    xT = np.ascontiguousarray(x.T)

    W1 = np.asarray(W1, dtype=np.float32)
    a1 = np.asarray(a1, dtype=np.float32)
    W2 = np.asarray(W2, dtype=np.float32)
    a2 = np.asarray(a2, dtype=np.float32)

    hs, nh = cfg.hstride, cfg.nhid
    w1t = np.zeros((cfg.f_in, cfg.w1cols), np.float32)
    for h in range(cfg.heads):
        w1t[:, h * hs: h * hs + nh] = W1[h].T
        va_s = W1[h].T @ a1[h, :nh]
        va_t = W1[h].T @ a1[h, nh:]
        w1t[:, h * hs + nh + 1] = va_s
        w1t[:, h * hs + nh + 2] = va_t
        w1t[:, h * hs + nh + 3] = 0.2 * va_t

    ncl = cfg.nclass
    w2t = np.zeros((cfg.fcat, cfg.w2cols), np.float32)
    w2t[:, 0:ncl] = W2.T
    va2_s = W2.T @ a2[:ncl]
    va2_t = W2.T @ a2[ncl:]
    w2t[:, ncl + 1] = va2_t
    w2t[:, ncl + 2] = 0.2 * va2_t
    w2t[:, ncl + 3] = va2_s

    in_maps = []
    max_dups = 0
    for c in range(cfg.ncores):
        blk = slice(c * s, (c + 1) * s)
        mine = (d_s >= c * s) & (d_s < (c + 1) * s)
        max_dups = max(max_dups, int(mine.sum()))
    dup_cap = cfg.dup_rounds * P
    assert max_dups <= dup_cap, f"{max_dups} dups > cap {dup_cap}"

    for c in range(cfg.ncores):
        blk = slice(c * s, (c + 1) * s)
        mine = np.nonzero((d_s >= c * s) & (d_s < (c + 1) * s))[0]
        k = len(mine)
        td = np.zeros((dup_cap, 1), np.int32)
        sd = np.zeros((dup_cap, 1), np.int32)
        dm = np.zeros((dup_cap, 1), np.float32)
        soh = np.zeros((dup_cap, s), np.float32)
        td[:k, 0] = d_t[mine]
        sd[:k, 0] = d_s[mine]
        dm[:k, 0] = d_m[mine]
        soh[np.arange(k), d_s[mine] - c * s] = 1.0
        in_maps.append({
            "xT": xT,
            "xTown": np.ascontiguousarray(xT[:, blk]),
            "w1t": w1t,
            "w2t": w2t,
            "maskT": np.ascontiguousarray(maskT[:, blk]),
            "tdup": td, "sdup": sd, "dmult": dm, "sonehot": soh,
        })
    return in_maps


def declare_io(nc, cfg: GATConfig):
    n, s = cfg.n, cfg.s
    dup_cap = cfg.dup_rounds * P
    io = {
        "xT": nc.dram_tensor("xT", [cfg.f_in, n], F32R, kind="ExternalInput").ap(),
        "xTown": nc.dram_tensor("xTown", [cfg.f_in, s], F32R, kind="ExternalInput").ap(),
        "w1t": nc.dram_tensor("w1t", [cfg.f_in, cfg.w1cols], F32R, kind="ExternalInput").ap(),
        "w2t": nc.dram_tensor("w2t", [cfg.fcat, cfg.w2cols], F32R, kind="ExternalInput").ap(),
        "maskT": nc.dram_tensor("maskT", [n, s], FP8, kind="ExternalInput").ap(),
        "tdup": nc.dram_tensor("tdup", [dup_cap, 1], I32, kind="ExternalInput").ap(),
        "sdup": nc.dram_tensor("sdup", [dup_cap, 1], I32, kind="ExternalInput").ap(),
        "dmult": nc.dram_tensor("dmult", [dup_cap, 1], F32, kind="ExternalInput").ap(),
        "sonehot": nc.dram_tensor("sonehot", [dup_cap, s], F32R, kind="ExternalInput").ap(),
        "out": nc.dram_tensor("out", [s, cfg.nclass], F32, kind="ExternalOutput").ap(),
    }
    return io


_ACT_PATCH_DIR = None


def install_patched_act_tables():
    """Repoint BASS_ACT_ROOT_JSON_PATH at a copy of the stock PWP tables in
    which exp's negative-domain buckets compute exp(0.2*x) instead of exp(x).
    The Exp activation then evaluates exp(leaky_relu(x)) in a single pass.
    (All other exp uses in this kernel feed arguments <= 0 scaled by 5, or
    split pos/neg, so they still compute a true exp.)"""
    global _ACT_PATCH_DIR
    import json
    import os
    import shutil
    import tempfile

    if _ACT_PATCH_DIR is not None:
        os.environ["BASS_ACT_ROOT_JSON_PATH"] = os.path.join(
            _ACT_PATCH_DIR, "act_info.json")
        return

    from neuronxcc.driver.Job import Job
    from neuronxcc.driver.jobs.support.FindActInfo import findActInfoFile

    src_json = findActInfoFile(Job.getPackageDir(), "gen3")
    src_dir = os.path.dirname(src_json)
    pwp_jsons = os.path.join(os.path.dirname(src_dir), "pwp_jsons")

    dst = tempfile.mkdtemp(prefix="act_lrelu_")
    for f in os.listdir(src_dir):
        shutil.copy(os.path.join(src_dir, f), os.path.join(dst, f))

    exp_def = json.load(open(os.path.join(pwp_jsons, "exp_400p.json")))
    neg_secs = []
    for e in exp_def["neg_exponents"]:
        for sct in e["exponent_sections"]:
            neg_secs.append(np.array(
                [sct["d0"]["int"], sct["d1"]["int"], sct["d2"]["int"],
                 sct["d3"]["int"], sct["x"]["int"]], dtype=np.uint32))

    info = json.load(open(os.path.join(dst, "act_info.json")))
    for st in info["act_func_sets"]:
        if "exp" not in st["act"]:
            continue
        path = os.path.join(dst, st["bkt_bin"])
        bkt = np.fromfile(path, dtype=np.uint32)
        view = np.lib.stride_tricks.sliding_window_view(bkt, 5)
        n_patched = 0
        for sec in neg_secs:
            m = np.where(np.all(view == sec, axis=1))[0]
            if len(m) != 1:
                continue
            i = int(m[0])
            x0 = float(sec[4:5].view(np.float32)[0])
            f = np.float32(math.exp(0.2 * x0))
            coef = np.array([f, 0.2 * f, 0.02 * f, (0.2 ** 3 / 6.0) * f],
                            dtype=np.float32)
            bkt[i:i + 4] = coef.view(np.uint32)
            n_patched += 1
        assert n_patched == len(neg_secs), (st["name"], n_patched)
        bkt.tofile(path)

    _ACT_PATCH_DIR = dst
    os.environ["BASS_ACT_ROOT_JSON_PATH"] = os.path.join(dst, "act_info.json")


def build_program(cfg: GATConfig):
    nc = bacc.Bacc("TRN2", target_bir_lowering=False, debug=False,
                   num_devices=cfg.ncores)
    io = declare_io(nc, cfg)
    with tile.TileContext(nc) as tc:
        build_gat_kernel(tc, cfg, io)
    nc.compile()
    return nc


_CACHE = {}


def kernel(x, edge_list, W1, b1, a1, W2, b2, a2, _trace=False, _tmpdir=None):
    cfg = GATConfig()
    assert np.asarray(b1).max() == 0 and np.asarray(b2).max() == 0
    in_maps = preprocess(cfg, np.asarray(x), np.asarray(edge_list),
                         np.asarray(W1), np.asarray(a1),
                         np.asarray(W2), np.asarray(a2))
    install_patched_act_tables()
    if cfg not in _CACHE:
        _CACHE[cfg] = build_program(cfg)
    nc = _CACHE[cfg]
    res = run_bass_kernel_spmd(
        nc, in_maps, core_ids=list(range(cfg.ncores)),
        trace=_trace, tmpdir=_tmpdir,
        **({"trace_cores": [0]} if _trace else {}))
    out = np.concatenate([r["out"] for r in res.results], axis=0)
    kernel._last_results = res
    return out.astype(np.float32)
